# revision 29
# baseline (speedup 1.0000x reference)
"""Local (windowed, causal) attention on 8 TRN2 NeuronCores.

Shapes (hardcoded): q,k,v [4, 8, 4096, 64] fp32, window=128, look_backward=1.
Sharding: merged batch*heads axis (32) -> 4 heads per core, data parallel.

The end-to-end call is dominated by the host<->device tunnel (~65 MB/s up,
~47 MB/s down), so the wire format is fp16 and all layout work happens on
device:
  - q, k ship as head-PAIR packed [pairs, T, 128] fp16 (cols = 2 heads x 64),
    one strided astype pass on host; the e-major transpose happens on device
    via the DMA XBAR transpose (16x128 tiles, ~14 ns/tile).
  - v ships natural [heads, T, 64] fp16; the softmax-denominator ones column
    is memset on device.
  - out comes back int8, quantized per output partition against its abs-max;
    the fp16 scales ride in the same tensor (bitcast), so the host dequant
    multiply by m/QMAX is the exact inverse of the device quant.
  - tri mask constant and the output zero-dummy live on device permanently.

Device algorithm per head pair, per key-window c (32 windows of 128 tokens):
  S^T = K_c^T . [Q_c | Q_{c+1}]      (one matmul per head; the two heads of a
                                      pair sit in PE row groups 0-63 / 64-127
                                      and overlap in the array)
  P^T = exp(scale * S^T)             (ACT, PSUM->SBUF, fp16)
  P^T[:, :128] *= tri                (GpSimd, causal mask on diagonal block)
  O_w += P^T_block . [V_c | 1]       (two matmuls accumulate the two key-window
                                      contributions per query window; the ones
                                      column accumulates the softmax denom)
  out_w = O_w[:, :64] * 1/O_w[:, 64] (DVE reciprocal + tensor_scalar_mul)
"""

import numpy as np

import concourse.bass as bass
import concourse.tile as tile
from concourse import bacc, mybir

B, H, T, E = 4, 8, 4096, 64
BH = B * H
WS = 128                      # window size
NW = T // WS                  # 32 windows per sequence
NCORES = 8
GPC = BH // NCORES            # 4 heads per core
NPAIR = GPC // 2              # 2 head pairs per core
SCALE = float(E) ** -0.5
F32 = mybir.dt.float32
F16 = mybir.dt.float16
I8 = mybir.dt.int8
QMAX = 126.0                  # int8 quant range (margin below 127 for safety)


def _emit(tc, qsrcs, ksrcs, vsrcs, tri, out):
    import contextlib

    nc = tc.nc
    Exp = mybir.ActivationFunctionType.Exp
    mult = mybir.AluOpType.mult

    with contextlib.ExitStack() as ctx:
        qk_pool = ctx.enter_context(tc.tile_pool(name="qk", bufs=2))
        v_pool = ctx.enter_context(tc.tile_pool(name="v", bufs=3))
        o_sb_pool = ctx.enter_context(tc.tile_pool(name="o_sb", bufs=3))
        p_pool = ctx.enter_context(tc.tile_pool(name="p", bufs=4))
        const_pool = ctx.enter_context(tc.tile_pool(name="const", bufs=1))
        s_pool = ctx.enter_context(tc.tile_pool(name="s", bufs=3, space="PSUM"))
        o_ps_pool = ctx.enter_context(tc.tile_pool(name="o_ps", bufs=5, space="PSUM"))
        r_pool = ctx.enter_context(tc.tile_pool(name="r", bufs=6))

        tri_sb = const_pool.tile([WS, WS], F16)
        nc.sync.dma_start(tri_sb[:], tri[:])

        for pair in range(NPAIR):
            # e-major Q/K for the pair via DMA XBAR transpose:
            # [T, 128] -> [128, T]; rows 0-63 head0's e, 64-127 head1's e.
            qT_t = qk_pool.tile([128, T], F16, tag="qT", name=f"qT_{pair}")
            nc.sync.dma_start_transpose(qT_t[:], qsrcs[pair][0])
            kT_t = qk_pool.tile([128, T], F16, tag="kT", name=f"kT_{pair}")
            nc.sync.dma_start_transpose(kT_t[:], ksrcs[pair][0])

            v_t, out_t, ot = [], [], [{}, {}]
            for gg in range(2):
                g = 2 * pair + gg
                vt = v_pool.tile([128, NW * 65], F16, tag="v", name=f"v_{pair}_{gg}")
                v3 = vt[:].rearrange("p (w e) -> p w e", e=65)
                nc.vector.memset(v3[:, :, 64:65], 1.0)
                nc.sync.dma_start(
                    v3[:, :, 0:64],
                    vsrcs[pair][gg].rearrange("(w p) e -> p w e", p=WS),
                )
                v_t.append(vt)
                outt = o_sb_pool.tile(
                    [128, NW * E], F16, tag="out", name=f"out_{pair}_{gg}"
                )
                out_t.append(outt)

            for c in range(NW):
                n = 256 if c < NW - 1 else 128
                s_t = []
                # both heads' QK^T back-to-back: disjoint PE row groups overlap
                for gg in range(2):
                    p0 = 64 * gg
                    st = s_pool.tile([128, 256], F32, tag="s", name=f"s_{pair}_{gg}_{c}")
                    nc.tensor.matmul(
                        st[:, :n],
                        lhsT=kT_t[p0 : p0 + 64, WS * c : WS * (c + 1)],
                        rhs=qT_t[p0 : p0 + 64, WS * c : WS * c + n],
                        start=True,
                        stop=True,
                    )
                    s_t.append(st)

                for gg in range(2):
                    st, vt, outt, od = s_t[gg], v_t[gg], out_t[gg], ot[gg]
                    p_t = p_pool.tile([128, 256], F16, tag="p", name=f"p_{pair}_{gg}_{c}")
                    nc.scalar.activation(p_t[:, :n], st[:, :n], Exp, scale=SCALE)
                    # causal mask on the diagonal block (keys j valid for i>=j)
                    nc.gpsimd.tensor_tensor(
                        p_t[:, :WS], p_t[:, :WS], tri_sb[:], op=mult
                    )

                    # PV for queries of window c (2nd contribution unless c==0)
                    if c == 0:
                        od[0] = o_ps_pool.tile(
                            [128, 65], F32, tag="o", name=f"o_{pair}_{gg}_0"
                        )
                    nc.tensor.matmul(
                        od[c][:],
                        lhsT=p_t[:, :WS],
                        rhs=vt[:, 65 * c : 65 * c + 65],
                        start=(c == 0),
                        stop=True,
                        skip_group_check=True,
                    )
                    # normalize window c -> SBUF out tile
                    rc = r_pool.tile([128, 1], F32, tag="rc", name=f"rc_{pair}_{gg}_{c}")
                    nc.vector.reciprocal(rc[:], od[c][:, 64:65])
                    nc.vector.tensor_scalar_mul(
                        outt[:, E * c : E * (c + 1)], od[c][:, 0:E], rc[:]
                    )
                    del od[c]

                    # PV for queries of window c+1 (1st contribution)
                    if c < NW - 1:
                        od[c + 1] = o_ps_pool.tile(
                            [128, 65], F32, tag="o", name=f"o_{pair}_{gg}_{c + 1}"
                        )
                        nc.tensor.matmul(
                            od[c + 1][:],
                            lhsT=p_t[:, WS : 2 * WS],
                            rhs=vt[:, 65 * c : 65 * c + 65],
                            start=True,
                            stop=False,
                            skip_group_check=True,
                        )

            for gg in range(2):
                g = 2 * pair + gg
                # int8-quantize against the per-partition abs-max; the fp16
                # scales ride along in the same int8 tensor (bitcast), so the
                # host multiply by m/QMAX is the exact inverse
                m_t = r_pool.tile([128, 1], F16, tag="m", name=f"m_{pair}_{gg}")
                nc.vector.tensor_reduce(
                    m_t[:],
                    out_t[gg][:],
                    axis=mybir.AxisListType.X,
                    op=mybir.AluOpType.max,
                    apply_absolute_value=True,
                )
                s_t = r_pool.tile([128, 1], F32, tag="sc", name=f"sc_{pair}_{gg}")
                nc.vector.reciprocal(s_t[:], m_t[:])
                nc.vector.tensor_scalar_mul(s_t[:], s_t[:], QMAX)
                oi8 = o_sb_pool.tile(
                    [128, NW * E], I8, tag="oi8", name=f"oi8_{pair}_{gg}"
                )
                nc.vector.tensor_scalar_mul(oi8[:], out_t[gg][:], s_t[:])
                nc.sync.dma_start(
                    out[g, : T * E].rearrange("(w p e) -> p w e", p=WS, e=E),
                    oi8[:].rearrange("p (w e) -> p w e", e=E),
                )
                nc.sync.dma_start(
                    out[g, T * E : T * E + 256].rearrange("(p b) -> p b", p=WS),
                    m_t[:].bitcast(I8),
                )


_CACHE = {}


def _build():
    if "nc" in _CACHE:
        return _CACHE["nc"]
    nc = bacc.Bacc(
        "TRN2",
        target_bir_lowering=False,
        debug=False,
        num_devices=NCORES,
    )
    q2 = nc.dram_tensor("q2", [NPAIR, T, 128], F16, kind="ExternalInput").ap()
    k2 = nc.dram_tensor("k2", [NPAIR, T, 128], F16, kind="ExternalInput").ap()
    v = nc.dram_tensor("v", [GPC, T, E], F16, kind="ExternalInput").ap()
    tri = nc.dram_tensor("tri", [WS, WS], F16, kind="ExternalInput").ap()
    # per head: T*E int8 payload + 256 bytes of bitcast fp16 scales
    out = nc.dram_tensor("out", [GPC, T * E + 256], I8, kind="ExternalOutput").ap()

    with tile.TileContext(nc) as tc:
        _emit(tc, [q2[0:1], q2[1:2]], [k2[0:1], k2[1:2]], [v[0:2], v[2:4]], tri, out)
    nc.compile()
    _CACHE["nc"] = nc
    return nc


def _tri_np():
    # tri[j, i] = 1.0 where query i >= key j (lower-left causal keep mask,
    # stored keys-in-partitions)
    return np.triu(np.ones((WS, WS), dtype=np.float16))


def _pack_qk(x):
    # [4, 8, T, E] fp32 -> [16 pairs, T, 128] fp16 (cols: head0 e | head1 e)
    x = np.asarray(x).reshape(BH // 2, 2, T, E)
    return x.transpose(0, 2, 1, 3).astype(np.float16).reshape(BH // 2, T, 2 * E)


def _prep_in_maps(q, k, v):
    """Per-core input dicts (used by the CoreSim gate in test.py)."""
    q2 = _pack_qk(q)
    k2 = _pack_qk(k)
    vm = np.asarray(v, dtype=np.float32).reshape(BH, T, E).astype(np.float16)
    tri = _tri_np()
    in_maps = []
    for i in range(NCORES):
        in_maps.append(
            {
                "q2": np.ascontiguousarray(q2[NPAIR * i : NPAIR * (i + 1)]),
                "k2": np.ascontiguousarray(k2[NPAIR * i : NPAIR * (i + 1)]),
                "v": np.ascontiguousarray(vm[GPC * i : GPC * (i + 1)]),
                "tri": tri,
            }
        )
    return in_maps


class _Runner:
    """Cached PJRT executor: traces/compiles the NEFF-wrapped jit once,
    keeps the tri constant and the output zero-dummy resident on device,
    and reuses everything across calls."""

    def __init__(self, nc):
        import jax
        from jax.experimental.shard_map import shard_map
        from jax.sharding import Mesh, PartitionSpec

        from concourse import bass2jax as b2j

        b2j.install_neuronx_cc_hook()
        self._jax = jax
        self.nc = nc
        part_name = nc.partition_id_tensor.name if nc.partition_id_tensor else None
        in_names, out_names, out_avals = [], [], []
        for alloc in nc.m.functions[0].allocations:
            if not isinstance(alloc, mybir.MemoryLocationSet):
                continue
            name = alloc.memorylocations[0].name
            if alloc.kind == "ExternalInput":
                if name != part_name:
                    in_names.append(name)
            elif alloc.kind == "ExternalOutput":
                out_names.append(name)
                shape = tuple(alloc.tensor_shape)
                dtype = mybir.dt.np(alloc.dtype)
                out_avals.append(jax.core.ShapedArray(shape, dtype))
        self.in_names, self.out_names = in_names, out_names
        n_params, n_outs = len(in_names), len(out_names)
        all_names = in_names + out_names
        if part_name is not None:
            all_names = all_names + [part_name]

        def _body(*args):
            operands = list(args)
            if part_name is not None:
                operands.append(b2j.partition_id_tensor())
            return tuple(
                b2j._bass_exec_p.bind(
                    *operands,
                    out_avals=tuple(out_avals),
                    in_names=tuple(all_names),
                    out_names=tuple(out_names),
                    lowering_input_output_aliases=(),
                    sim_require_finite=True,
                    sim_require_nnan=True,
                    nc=nc,
                )
            )

        devices = jax.devices()[:NCORES]
        mesh = Mesh(np.asarray(devices), ("core",))
        self.mesh = mesh
        self.sharding = jax.sharding.NamedSharding(mesh, PartitionSpec("core"))
        self.jitted = jax.jit(
            shard_map(
                _body,
                mesh=mesh,
                in_specs=(PartitionSpec("core"),) * (n_params + n_outs),
                out_specs=(PartitionSpec("core"),) * n_outs,
                check_rep=False,
            ),
            keep_unused=True,
        )
        assert self.in_names == ["q2", "k2", "v", "tri"], self.in_names
        assert self.out_names == ["out"], self.out_names
        # persistent device-resident constants (transferred once)
        self.d_tri = jax.device_put(np.tile(_tri_np(), (NCORES, 1)), self.sharding)
        self.d_zero_out = jax.device_put(
            np.zeros((NCORES * GPC, T * E + 256), np.int8), self.sharding
        )
        from concurrent.futures import ThreadPoolExecutor

        self.pool = ThreadPoolExecutor(4)
        self.q2_buf = np.empty((BH // 2, T, 2 * E), np.float16)
        self.k2_buf = np.empty((BH // 2, T, 2 * E), np.float16)
        self.v16_buf = np.empty((BH, T, E), np.float16)

    def put(self, arr):
        return self._jax.device_put(arr, self.sharding)

    def pack_qk_fast(self, x, buf):
        # [4, 8, T, E] fp32 -> [16 pairs, T, 128] fp16, 4 threads
        xr = np.asarray(x).reshape(BH // 2, 2, T, E)

        def fill(i0, i1):
            buf[i0:i1, :, :E] = xr[i0:i1, 0]
            buf[i0:i1, :, E:] = xr[i0:i1, 1]

        futs = [self.pool.submit(fill, 4 * i, 4 * i + 4) for i in range(4)]
        for f in futs:
            f.result()
        return buf

    def call_device(self, dq2, dk2, dv):
        (out,) = self.jitted(dq2, dk2, dv, self.d_tri, self.d_zero_out)
        return out


def _get_runner():
    if "runner" not in _CACHE:
        _CACHE["runner"] = _Runner(_build())
    return _CACHE["runner"]


def _dequant_into(res, dst):
    # res [nh, T*E+256] int8 -> dst [nh, T, E] fp32 (single fused pass)
    nh = res.shape[0]
    m = res[:, T * E :].reshape(nh, WS, 2).copy().view(np.float16)
    np.multiply(
        res[:, : T * E].reshape(nh, NW, WS, E),
        m.astype(np.float32).reshape(nh, 1, WS, 1) / QMAX,
        out=dst.reshape(nh, NW, WS, E),
    )


def kernel(q, k, v):
    r = _get_runner()
    # pack + upload; device_put dispatches async so the next pack overlaps
    # the previous transfer
    dq = r.put(r.pack_qk_fast(q, r.q2_buf))
    dk = r.put(r.pack_qk_fast(k, r.k2_buf))
    np.copyto(r.v16_buf, np.asarray(v).reshape(BH, T, E))
    dv = r.put(r.v16_buf)
    out = r.call_device(dq, dk, dv)
    full = np.empty((BH, T, E), np.float32)
    _dequant_into(np.asarray(out), full)
    return full.reshape(B, H, T, E)


def run(q, k, v, **kw):
    return kernel(q, k, v), None


# revision 36
# speedup vs baseline: 1.1743x; 1.1743x over previous
"""Local (windowed, causal) attention on 8 TRN2 NeuronCores.

Shapes (hardcoded): q,k,v [4, 8, 4096, 64] fp32, window=128, look_backward=1.
Sharding: merged batch*heads axis (32) -> 4 heads per core, data parallel.

The end-to-end call is dominated by the host<->device tunnel (~65 MB/s up,
~47 MB/s down), so the wire format is fp16 and all layout work happens on
device:
  - q, k ship as head-PAIR packed [pairs, T, 128] fp16 (cols = 2 heads x 64),
    one strided astype pass on host; the e-major transpose happens on device
    via the DMA XBAR transpose (16x128 tiles, ~14 ns/tile).
  - v ships natural [heads, T, 64] fp16; the softmax-denominator ones column
    is memset on device.
  - out comes back int8, quantized per output partition against its abs-max;
    the fp16 scales ride in the same tensor (bitcast), so the host dequant
    multiply by m/QMAX is the exact inverse of the device quant.
  - tri mask constant and the output zero-dummy live on device permanently.

Device algorithm per head pair, per key-window c (32 windows of 128 tokens):
  S^T = K_c^T . [Q_c | Q_{c+1}]      (one matmul per head; the two heads of a
                                      pair sit in PE row groups 0-63 / 64-127
                                      and overlap in the array)
  P^T = exp(scale * S^T)             (ACT, PSUM->SBUF, fp16)
  P^T[:, :128] *= tri                (GpSimd, causal mask on diagonal block)
  O_w += P^T_block . [V_c | 1]       (two matmuls accumulate the two key-window
                                      contributions per query window; the ones
                                      column accumulates the softmax denom)
  out_w = O_w[:, :64] * 1/O_w[:, 64] (DVE reciprocal + tensor_scalar_mul)
"""

import numpy as np

import concourse.bass as bass
import concourse.tile as tile
from concourse import bacc, mybir

B, H, T, E = 4, 8, 4096, 64
BH = B * H
WS = 128                      # window size
NW = T // WS                  # 32 windows per sequence
NCORES = 8
GPC = BH // NCORES            # 4 heads per core
NPAIR = GPC // 2              # 2 head pairs per core
SCALE = float(E) ** -0.5
F32 = mybir.dt.float32
F16 = mybir.dt.float16
I8 = mybir.dt.int8
QMAX = 126.0                  # int8 quant range (margin below 127 for safety)


def _emit(tc, qsrcs, ksrcs, vsrcs, tri, out):
    import contextlib

    nc = tc.nc
    Exp = mybir.ActivationFunctionType.Exp
    mult = mybir.AluOpType.mult

    with contextlib.ExitStack() as ctx:
        qk_pool = ctx.enter_context(tc.tile_pool(name="qk", bufs=2))
        v_pool = ctx.enter_context(tc.tile_pool(name="v", bufs=3))
        vi_pool = ctx.enter_context(tc.tile_pool(name="vi", bufs=3))
        o_sb_pool = ctx.enter_context(tc.tile_pool(name="o_sb", bufs=3))
        p_pool = ctx.enter_context(tc.tile_pool(name="p", bufs=4))
        const_pool = ctx.enter_context(tc.tile_pool(name="const", bufs=1))
        s_pool = ctx.enter_context(tc.tile_pool(name="s", bufs=3, space="PSUM"))
        o_ps_pool = ctx.enter_context(tc.tile_pool(name="o_ps", bufs=5, space="PSUM"))
        r_pool = ctx.enter_context(tc.tile_pool(name="r", bufs=6))

        tri_sb = const_pool.tile([WS, WS], F16)
        nc.sync.dma_start(tri_sb[:], tri[:])

        for pair in range(NPAIR):
            # e-major Q/K for the pair via DMA XBAR transpose:
            # [T, 128] -> [128, T]; rows 0-63 head0's e, 64-127 head1's e.
            qT_t = qk_pool.tile([128, T], F16, tag="qT", name=f"qT_{pair}")
            nc.sync.dma_start_transpose(qT_t[:], qsrcs[pair][0])
            kT_t = qk_pool.tile([128, T], F16, tag="kT", name=f"kT_{pair}")
            nc.sync.dma_start_transpose(kT_t[:], ksrcs[pair][0])

            v_t, out_t, ot = [], [], [{}, {}]
            for gg in range(2):
                g = 2 * pair + gg
                # v arrives int8 (per-head scale embedded as bitcast fp32 in
                # the tail); dequantize to fp16 on device
                vi8 = vi_pool.tile([128, NW * E], I8, tag="vi", name=f"vi_{pair}_{gg}")
                nc.sync.dma_start(
                    vi8[:].rearrange("p (w e) -> p w e", e=E),
                    vsrcs[pair][gg][: T * E].rearrange("(w p e) -> p w e", p=WS, e=E),
                )
                vs_t = r_pool.tile([128, 1], F32, tag="vs", name=f"vs_{pair}_{gg}")
                nc.sync.dma_start(
                    vs_t[:],
                    vsrcs[pair][gg][T * E : T * E + 512]
                    .rearrange("(p b) -> p b", p=WS)
                    .bitcast(F32),
                )
                vt = v_pool.tile([128, NW * 65], F16, tag="v", name=f"v_{pair}_{gg}")
                v3 = vt[:].rearrange("p (w e) -> p w e", e=65)
                nc.vector.memset(v3[:, :, 64:65], 1.0)
                nc.vector.tensor_scalar_mul(
                    v3[:, :, 0:64],
                    vi8[:].rearrange("p (w e) -> p w e", e=E),
                    vs_t[:],
                )
                v_t.append(vt)
                outt = o_sb_pool.tile(
                    [128, NW * E], F16, tag="out", name=f"out_{pair}_{gg}"
                )
                out_t.append(outt)

            for c in range(NW):
                n = 256 if c < NW - 1 else 128
                s_t = []
                # both heads' QK^T back-to-back: disjoint PE row groups overlap
                for gg in range(2):
                    p0 = 64 * gg
                    st = s_pool.tile([128, 256], F32, tag="s", name=f"s_{pair}_{gg}_{c}")
                    nc.tensor.matmul(
                        st[:, :n],
                        lhsT=kT_t[p0 : p0 + 64, WS * c : WS * (c + 1)],
                        rhs=qT_t[p0 : p0 + 64, WS * c : WS * c + n],
                        start=True,
                        stop=True,
                    )
                    s_t.append(st)

                for gg in range(2):
                    st, vt, outt, od = s_t[gg], v_t[gg], out_t[gg], ot[gg]
                    p_t = p_pool.tile([128, 256], F16, tag="p", name=f"p_{pair}_{gg}_{c}")
                    nc.scalar.activation(p_t[:, :n], st[:, :n], Exp, scale=SCALE)
                    # causal mask on the diagonal block (keys j valid for i>=j)
                    nc.gpsimd.tensor_tensor(
                        p_t[:, :WS], p_t[:, :WS], tri_sb[:], op=mult
                    )

                    # PV for queries of window c (2nd contribution unless c==0)
                    if c == 0:
                        od[0] = o_ps_pool.tile(
                            [128, 65], F32, tag="o", name=f"o_{pair}_{gg}_0"
                        )
                    nc.tensor.matmul(
                        od[c][:],
                        lhsT=p_t[:, :WS],
                        rhs=vt[:, 65 * c : 65 * c + 65],
                        start=(c == 0),
                        stop=True,
                        skip_group_check=True,
                    )
                    # normalize window c -> SBUF out tile
                    rc = r_pool.tile([128, 1], F32, tag="rc", name=f"rc_{pair}_{gg}_{c}")
                    nc.vector.reciprocal(rc[:], od[c][:, 64:65])
                    nc.vector.tensor_scalar_mul(
                        outt[:, E * c : E * (c + 1)], od[c][:, 0:E], rc[:]
                    )
                    del od[c]

                    # PV for queries of window c+1 (1st contribution)
                    if c < NW - 1:
                        od[c + 1] = o_ps_pool.tile(
                            [128, 65], F32, tag="o", name=f"o_{pair}_{gg}_{c + 1}"
                        )
                        nc.tensor.matmul(
                            od[c + 1][:],
                            lhsT=p_t[:, WS : 2 * WS],
                            rhs=vt[:, 65 * c : 65 * c + 65],
                            start=True,
                            stop=False,
                            skip_group_check=True,
                        )

            for gg in range(2):
                g = 2 * pair + gg
                # int8-quantize against the per-partition abs-max; the fp16
                # scales ride along in the same int8 tensor (bitcast), so the
                # host multiply by m/QMAX is the exact inverse
                m_t = r_pool.tile([128, 1], F16, tag="m", name=f"m_{pair}_{gg}")
                nc.vector.tensor_reduce(
                    m_t[:],
                    out_t[gg][:],
                    axis=mybir.AxisListType.X,
                    op=mybir.AluOpType.max,
                    apply_absolute_value=True,
                )
                s_t = r_pool.tile([128, 1], F32, tag="sc", name=f"sc_{pair}_{gg}")
                nc.vector.reciprocal(s_t[:], m_t[:])
                nc.vector.tensor_scalar_mul(s_t[:], s_t[:], QMAX)
                oi8 = o_sb_pool.tile(
                    [128, NW * E], I8, tag="oi8", name=f"oi8_{pair}_{gg}"
                )
                nc.vector.tensor_scalar_mul(oi8[:], out_t[gg][:], s_t[:])
                nc.sync.dma_start(
                    out[g, : T * E].rearrange("(w p e) -> p w e", p=WS, e=E),
                    oi8[:].rearrange("p (w e) -> p w e", e=E),
                )
                nc.sync.dma_start(
                    out[g, T * E : T * E + 256].rearrange("(p b) -> p b", p=WS),
                    m_t[:].bitcast(I8),
                )


_CACHE = {}


def _build():
    if "nc" in _CACHE:
        return _CACHE["nc"]
    nc = bacc.Bacc(
        "TRN2",
        target_bir_lowering=False,
        debug=False,
        num_devices=NCORES,
    )
    q2 = nc.dram_tensor("q2", [NPAIR, T, 128], F16, kind="ExternalInput").ap()
    k2 = nc.dram_tensor("k2", [NPAIR, T, 128], F16, kind="ExternalInput").ap()
    # per head: T*E int8 payload + 512 bytes of bitcast fp32 dequant scale
    # (the per-head scale replicated over the 128 partitions)
    v = nc.dram_tensor("v", [GPC, T * E + 512], I8, kind="ExternalInput").ap()
    tri = nc.dram_tensor("tri", [WS, WS], F16, kind="ExternalInput").ap()
    # per head: T*E int8 payload + 256 bytes of bitcast fp16 scales
    out = nc.dram_tensor("out", [GPC, T * E + 256], I8, kind="ExternalOutput").ap()

    with tile.TileContext(nc) as tc:
        _emit(tc, [q2[0:1], q2[1:2]], [k2[0:1], k2[1:2]], [v[0:2], v[2:4]], tri, out)
    nc.compile()
    _CACHE["nc"] = nc
    return nc


def _tri_np():
    # tri[j, i] = 1.0 where query i >= key j (lower-left causal keep mask,
    # stored keys-in-partitions)
    return np.triu(np.ones((WS, WS), dtype=np.float16))


def _pack_qk(x):
    # [4, 8, T, E] fp32 -> [16 pairs, T, 128] fp16 (cols: head0 e | head1 e)
    x = np.asarray(x).reshape(BH // 2, 2, T, E)
    return x.transpose(0, 2, 1, 3).astype(np.float16).reshape(BH // 2, T, 2 * E)


def _quant_v(vr, vbuf=None, vtmp=None):
    # vr [32, T, E] fp32 -> [32, T*E+512] int8: round(v*QMAX/m) payload plus
    # the fp32 dequant scale m/QMAX replicated x128, bitcast into the tail
    if vbuf is None:
        vbuf = np.empty((BH, T * E + 512), np.int8)
    if vtmp is None:
        vtmp = np.empty((BH, T, E), np.float32)
    np.abs(vr, out=vtmp)
    m = vtmp.max(axis=(1, 2))  # [32] per-head abs-max
    np.multiply(vr, (QMAX / m)[:, None, None], out=vtmp)
    np.rint(vtmp, out=vtmp)
    vbuf[:, : T * E] = vtmp.reshape(BH, T * E)  # exact cast of integral fp32
    vbuf[:, T * E :] = np.repeat(
        (m / QMAX).astype(np.float32)[:, None], WS, axis=1
    ).view(np.int8)
    return vbuf


def _prep_in_maps(q, k, v):
    """Per-core input dicts (used by the CoreSim gate in test.py)."""
    q2 = _pack_qk(q)
    k2 = _pack_qk(k)
    vq = _quant_v(np.asarray(v, dtype=np.float32).reshape(BH, T, E))
    tri = _tri_np()
    in_maps = []
    for i in range(NCORES):
        in_maps.append(
            {
                "q2": np.ascontiguousarray(q2[NPAIR * i : NPAIR * (i + 1)]),
                "k2": np.ascontiguousarray(k2[NPAIR * i : NPAIR * (i + 1)]),
                "v": np.ascontiguousarray(vq[GPC * i : GPC * (i + 1)]),
                "tri": tri,
            }
        )
    return in_maps


class _Runner:
    """Cached PJRT executor: traces/compiles the NEFF-wrapped jit once,
    keeps the tri constant and the output zero-dummy resident on device,
    and reuses everything across calls."""

    def __init__(self, nc):
        import jax
        from jax.experimental.shard_map import shard_map
        from jax.sharding import Mesh, PartitionSpec

        from concourse import bass2jax as b2j

        b2j.install_neuronx_cc_hook()
        self._jax = jax
        self.nc = nc
        part_name = nc.partition_id_tensor.name if nc.partition_id_tensor else None
        in_names, out_names, out_avals = [], [], []
        for alloc in nc.m.functions[0].allocations:
            if not isinstance(alloc, mybir.MemoryLocationSet):
                continue
            name = alloc.memorylocations[0].name
            if alloc.kind == "ExternalInput":
                if name != part_name:
                    in_names.append(name)
            elif alloc.kind == "ExternalOutput":
                out_names.append(name)
                shape = tuple(alloc.tensor_shape)
                dtype = mybir.dt.np(alloc.dtype)
                out_avals.append(jax.core.ShapedArray(shape, dtype))
        self.in_names, self.out_names = in_names, out_names
        n_params, n_outs = len(in_names), len(out_names)
        all_names = in_names + out_names
        if part_name is not None:
            all_names = all_names + [part_name]

        def _body(*args):
            operands = list(args)
            if part_name is not None:
                operands.append(b2j.partition_id_tensor())
            return tuple(
                b2j._bass_exec_p.bind(
                    *operands,
                    out_avals=tuple(out_avals),
                    in_names=tuple(all_names),
                    out_names=tuple(out_names),
                    lowering_input_output_aliases=(),
                    sim_require_finite=True,
                    sim_require_nnan=True,
                    nc=nc,
                )
            )

        devices = jax.devices()[:NCORES]
        mesh = Mesh(np.asarray(devices), ("core",))
        self.mesh = mesh
        self.sharding = jax.sharding.NamedSharding(mesh, PartitionSpec("core"))
        self.jitted = jax.jit(
            shard_map(
                _body,
                mesh=mesh,
                in_specs=(PartitionSpec("core"),) * (n_params + n_outs),
                out_specs=(PartitionSpec("core"),) * n_outs,
                check_rep=False,
            ),
            keep_unused=True,
        )
        assert self.in_names == ["q2", "k2", "v", "tri"], self.in_names
        assert self.out_names == ["out"], self.out_names
        # persistent device-resident constants (transferred once)
        self.d_tri = jax.device_put(np.tile(_tri_np(), (NCORES, 1)), self.sharding)
        self.d_zero_out = jax.device_put(
            np.zeros((NCORES * GPC, T * E + 256), np.int8), self.sharding
        )
        from concurrent.futures import ThreadPoolExecutor

        self.pool = ThreadPoolExecutor(4)
        self.q2_buf = np.empty((BH // 2, T, 2 * E), np.float16)
        self.k2_buf = np.empty((BH // 2, T, 2 * E), np.float16)
        self.vbuf = np.empty((BH, T * E + 512), np.int8)
        self.vtmp = np.empty((BH, T, E), np.float32)

    def put(self, arr):
        return self._jax.device_put(arr, self.sharding)

    def pack_qk_fast(self, x, buf):
        # [4, 8, T, E] fp32 -> [16 pairs, T, 128] fp16, 4 threads
        xr = np.asarray(x).reshape(BH // 2, 2, T, E)

        def fill(i0, i1):
            buf[i0:i1, :, :E] = xr[i0:i1, 0]
            buf[i0:i1, :, E:] = xr[i0:i1, 1]

        futs = [self.pool.submit(fill, 4 * i, 4 * i + 4) for i in range(4)]
        for f in futs:
            f.result()
        return buf

    def call_device(self, dq2, dk2, dv):
        (out,) = self.jitted(dq2, dk2, dv, self.d_tri, self.d_zero_out)
        return out


def _get_runner():
    if "runner" not in _CACHE:
        _CACHE["runner"] = _Runner(_build())
    return _CACHE["runner"]


def _dequant_into(res, dst):
    # res [nh, T*E+256] int8 -> dst [nh, T, E] fp32 (single fused pass)
    nh = res.shape[0]
    m = res[:, T * E :].reshape(nh, WS, 2).copy().view(np.float16)
    np.multiply(
        res[:, : T * E].reshape(nh, NW, WS, E),
        m.astype(np.float32).reshape(nh, 1, WS, 1) / QMAX,
        out=dst.reshape(nh, NW, WS, E),
    )


def kernel(q, k, v):
    r = _get_runner()
    # pack + upload; device_put dispatches async so the next pack overlaps
    # the previous transfer
    dq = r.put(r.pack_qk_fast(q, r.q2_buf))
    dk = r.put(r.pack_qk_fast(k, r.k2_buf))
    vr = np.asarray(v, dtype=np.float32).reshape(BH, T, E)
    dv = r.put(_quant_v(vr, r.vbuf, r.vtmp))
    out = r.call_device(dq, dk, dv)
    full = np.empty((BH, T, E), np.float32)
    _dequant_into(np.asarray(out), full)
    return full.reshape(B, H, T, E)


def run(q, k, v, **kw):
    return kernel(q, k, v), None


# revision 39
# speedup vs baseline: 1.2563x; 1.0698x over previous
"""Local (windowed, causal) attention on 8 TRN2 NeuronCores.

Shapes (hardcoded): q,k,v [4, 8, 4096, 64] fp32, window=128, look_backward=1.
Sharding: merged batch*heads axis (32) -> 4 heads per core, data parallel.

The end-to-end call is dominated by the host<->device tunnel (~65 MB/s up,
~47 MB/s down), so the wire format is fp16 and all layout work happens on
device:
  - q, k ship as head-PAIR packed [pairs, T, 128] fp16 (cols = 2 heads x 64),
    one strided astype pass on host; the e-major transpose happens on device
    via the DMA XBAR transpose (16x128 tiles, ~14 ns/tile).
  - v ships natural [heads, T, 64] fp16; the softmax-denominator ones column
    is memset on device.
  - out comes back int8, quantized per output partition against its abs-max;
    the fp16 scales ride in the same tensor (bitcast), so the host dequant
    multiply by m/QMAX is the exact inverse of the device quant.
  - tri mask constant and the output zero-dummy live on device permanently.

Device algorithm per head pair, per key-window c (32 windows of 128 tokens):
  S^T = K_c^T . [Q_c | Q_{c+1}]      (one matmul per head; the two heads of a
                                      pair sit in PE row groups 0-63 / 64-127
                                      and overlap in the array)
  P^T = exp(scale * S^T)             (ACT, PSUM->SBUF, fp16)
  P^T[:, :128] *= tri                (GpSimd, causal mask on diagonal block)
  O_w += P^T_block . [V_c | 1]       (two matmuls accumulate the two key-window
                                      contributions per query window; the ones
                                      column accumulates the softmax denom)
  out_w = O_w[:, :64] * 1/O_w[:, 64] (DVE reciprocal + tensor_scalar_mul)
"""

import numpy as np

import concourse.bass as bass
import concourse.tile as tile
from concourse import bacc, mybir

B, H, T, E = 4, 8, 4096, 64
BH = B * H
WS = 128                      # window size
NW = T // WS                  # 32 windows per sequence
NCORES = 8
GPC = BH // NCORES            # 4 heads per core
NPAIR = GPC // 2              # 2 head pairs per core
SCALE = float(E) ** -0.5
F32 = mybir.dt.float32
F16 = mybir.dt.float16
I8 = mybir.dt.int8
QMAX = 126.0                  # int8 quant range (margin below 127 for safety)


def _emit(tc, qsrcs, ksrcs, vsrcs, tri, out):
    import contextlib

    nc = tc.nc
    Exp = mybir.ActivationFunctionType.Exp
    mult = mybir.AluOpType.mult

    with contextlib.ExitStack() as ctx:
        qk_pool = ctx.enter_context(tc.tile_pool(name="qk", bufs=2))
        v_pool = ctx.enter_context(tc.tile_pool(name="v", bufs=3))
        vi_pool = ctx.enter_context(tc.tile_pool(name="vi", bufs=3))
        o_sb_pool = ctx.enter_context(tc.tile_pool(name="o_sb", bufs=3))
        p_pool = ctx.enter_context(tc.tile_pool(name="p", bufs=4))
        const_pool = ctx.enter_context(tc.tile_pool(name="const", bufs=1))
        s_pool = ctx.enter_context(tc.tile_pool(name="s", bufs=3, space="PSUM"))
        o_ps_pool = ctx.enter_context(tc.tile_pool(name="o_ps", bufs=5, space="PSUM"))
        r_pool = ctx.enter_context(tc.tile_pool(name="r", bufs=6))

        tri_sb = const_pool.tile([WS, WS], F16)
        nc.sync.dma_start(tri_sb[:], tri[:])

        for pair in range(NPAIR):
            # e-major Q/K for the pair via DMA XBAR transpose:
            # [T, 128] -> [128, T]; rows 0-63 head0's e, 64-127 head1's e.
            qT_t = qk_pool.tile([128, T], F16, tag="qT", name=f"qT_{pair}")
            nc.sync.dma_start_transpose(qT_t[:], qsrcs[pair][0])
            kT_t = qk_pool.tile([128, T], F16, tag="kT", name=f"kT_{pair}")
            nc.sync.dma_start_transpose(kT_t[:], ksrcs[pair][0])

            v_t, out_t, ot = [], [], [{}, {}]
            for gg in range(2):
                g = 2 * pair + gg
                # v arrives int8 (per-head scale embedded as bitcast fp32 in
                # the tail); dequantize to fp16 on device
                vi8 = vi_pool.tile([128, NW * E], I8, tag="vi", name=f"vi_{pair}_{gg}")
                nc.sync.dma_start(
                    vi8[:].rearrange("p (w e) -> p w e", e=E),
                    vsrcs[pair][gg][: T * E].rearrange("(w p e) -> p w e", p=WS, e=E),
                )
                vs_t = r_pool.tile([128, 1], F32, tag="vs", name=f"vs_{pair}_{gg}")
                nc.sync.dma_start(
                    vs_t[:],
                    vsrcs[pair][gg][T * E : T * E + 512]
                    .rearrange("(p b) -> p b", p=WS)
                    .bitcast(F32),
                )
                vt = v_pool.tile([128, NW * 65], F16, tag="v", name=f"v_{pair}_{gg}")
                v3 = vt[:].rearrange("p (w e) -> p w e", e=65)
                nc.vector.memset(v3[:, :, 64:65], 1.0)
                nc.vector.tensor_scalar_mul(
                    v3[:, :, 0:64],
                    vi8[:].rearrange("p (w e) -> p w e", e=E),
                    vs_t[:],
                )
                v_t.append(vt)
                outt = o_sb_pool.tile(
                    [128, NW * E], F16, tag="out", name=f"out_{pair}_{gg}"
                )
                out_t.append(outt)

            for c in range(NW):
                n = 256 if c < NW - 1 else 128
                s_t = []
                # both heads' QK^T back-to-back: disjoint PE row groups overlap
                for gg in range(2):
                    p0 = 64 * gg
                    st = s_pool.tile([128, 256], F32, tag="s", name=f"s_{pair}_{gg}_{c}")
                    nc.tensor.matmul(
                        st[:, :n],
                        lhsT=kT_t[p0 : p0 + 64, WS * c : WS * (c + 1)],
                        rhs=qT_t[p0 : p0 + 64, WS * c : WS * c + n],
                        start=True,
                        stop=True,
                    )
                    s_t.append(st)

                for gg in range(2):
                    st, vt, outt, od = s_t[gg], v_t[gg], out_t[gg], ot[gg]
                    p_t = p_pool.tile([128, 256], F16, tag="p", name=f"p_{pair}_{gg}_{c}")
                    nc.scalar.activation(p_t[:, :n], st[:, :n], Exp, scale=SCALE)
                    # causal mask on the diagonal block (keys j valid for i>=j)
                    nc.gpsimd.tensor_tensor(
                        p_t[:, :WS], p_t[:, :WS], tri_sb[:], op=mult
                    )

                    # PV for queries of window c (2nd contribution unless c==0)
                    if c == 0:
                        od[0] = o_ps_pool.tile(
                            [128, 65], F32, tag="o", name=f"o_{pair}_{gg}_0"
                        )
                    nc.tensor.matmul(
                        od[c][:],
                        lhsT=p_t[:, :WS],
                        rhs=vt[:, 65 * c : 65 * c + 65],
                        start=(c == 0),
                        stop=True,
                        skip_group_check=True,
                    )
                    # normalize window c -> SBUF out tile
                    rc = r_pool.tile([128, 1], F32, tag="rc", name=f"rc_{pair}_{gg}_{c}")
                    nc.vector.reciprocal(rc[:], od[c][:, 64:65])
                    nc.vector.tensor_scalar_mul(
                        outt[:, E * c : E * (c + 1)], od[c][:, 0:E], rc[:]
                    )
                    del od[c]

                    # PV for queries of window c+1 (1st contribution)
                    if c < NW - 1:
                        od[c + 1] = o_ps_pool.tile(
                            [128, 65], F32, tag="o", name=f"o_{pair}_{gg}_{c + 1}"
                        )
                        nc.tensor.matmul(
                            od[c + 1][:],
                            lhsT=p_t[:, WS : 2 * WS],
                            rhs=vt[:, 65 * c : 65 * c + 65],
                            start=True,
                            stop=False,
                            skip_group_check=True,
                        )

            for gg in range(2):
                g = 2 * pair + gg
                # int8-quantize against the per-partition abs-max; the fp16
                # scales ride along in the same int8 tensor (bitcast), so the
                # host multiply by m/QMAX is the exact inverse
                m_t = r_pool.tile([128, 1], F16, tag="m", name=f"m_{pair}_{gg}")
                nc.vector.tensor_reduce(
                    m_t[:],
                    out_t[gg][:],
                    axis=mybir.AxisListType.X,
                    op=mybir.AluOpType.max,
                    apply_absolute_value=True,
                )
                s_t = r_pool.tile([128, 1], F32, tag="sc", name=f"sc_{pair}_{gg}")
                nc.vector.reciprocal(s_t[:], m_t[:])
                nc.vector.tensor_scalar_mul(s_t[:], s_t[:], QMAX)
                oi8 = o_sb_pool.tile(
                    [128, NW * E], I8, tag="oi8", name=f"oi8_{pair}_{gg}"
                )
                nc.vector.tensor_scalar_mul(oi8[:], out_t[gg][:], s_t[:])
                nc.sync.dma_start(
                    out[g, : T * E].rearrange("(w p e) -> p w e", p=WS, e=E),
                    oi8[:].rearrange("p (w e) -> p w e", e=E),
                )
                nc.sync.dma_start(
                    out[g, T * E : T * E + 256].rearrange("(p b) -> p b", p=WS),
                    m_t[:].bitcast(I8),
                )


_CACHE = {}


def _build():
    if "nc" in _CACHE:
        return _CACHE["nc"]
    nc = bacc.Bacc(
        "TRN2",
        target_bir_lowering=False,
        debug=False,
        num_devices=NCORES,
    )
    q2 = nc.dram_tensor("q2", [NPAIR, T, 128], F16, kind="ExternalInput").ap()
    k2 = nc.dram_tensor("k2", [NPAIR, T, 128], F16, kind="ExternalInput").ap()
    # per head: T*E int8 payload + 512 bytes of bitcast fp32 dequant scale
    # (the per-head scale replicated over the 128 partitions)
    v = nc.dram_tensor("v", [GPC, T * E + 512], I8, kind="ExternalInput").ap()
    tri = nc.dram_tensor("tri", [WS, WS], F16, kind="ExternalInput").ap()
    # per head: T*E int8 payload + 256 bytes of bitcast fp16 scales
    out = nc.dram_tensor("out", [GPC, T * E + 256], I8, kind="ExternalOutput").ap()

    with tile.TileContext(nc) as tc:
        _emit(tc, [q2[0:1], q2[1:2]], [k2[0:1], k2[1:2]], [v[0:2], v[2:4]], tri, out)
    nc.compile()
    _CACHE["nc"] = nc
    return nc


def _tri_np():
    # tri[j, i] = 1.0 where query i >= key j (lower-left causal keep mask,
    # stored keys-in-partitions)
    return np.triu(np.ones((WS, WS), dtype=np.float16))


def _pack_qk(x):
    # [4, 8, T, E] fp32 -> [16 pairs, T, 128] fp16 (cols: head0 e | head1 e)
    x = np.asarray(x).reshape(BH // 2, 2, T, E)
    return x.transpose(0, 2, 1, 3).astype(np.float16).reshape(BH // 2, T, 2 * E)


def _quant_v(vr, vbuf=None, vtmp=None):
    # vr [nh, T, E] fp32 -> [nh, T*E+512] int8: round(v*QMAX/m) payload plus
    # the fp32 dequant scale m/QMAX replicated x128, bitcast into the tail
    nh = vr.shape[0]
    if vbuf is None:
        vbuf = np.empty((nh, T * E + 512), np.int8)
    if vtmp is None:
        vtmp = np.empty((nh, T, E), np.float32)
    np.abs(vr, out=vtmp)
    m = vtmp.max(axis=(1, 2))  # [nh] per-head abs-max
    np.multiply(vr, (QMAX / m)[:, None, None], out=vtmp)
    np.rint(vtmp, out=vtmp)
    vbuf[:, : T * E] = vtmp.reshape(nh, T * E)  # exact cast of integral fp32
    vbuf[:, T * E :] = np.repeat(
        (m / QMAX).astype(np.float32)[:, None], WS, axis=1
    ).view(np.int8)
    return vbuf


def _prep_in_maps(q, k, v):
    """Per-core input dicts (used by the CoreSim gate in test.py)."""
    q2 = _pack_qk(q)
    k2 = _pack_qk(k)
    vq = _quant_v(np.asarray(v, dtype=np.float32).reshape(BH, T, E))
    tri = _tri_np()
    in_maps = []
    for i in range(NCORES):
        in_maps.append(
            {
                "q2": np.ascontiguousarray(q2[NPAIR * i : NPAIR * (i + 1)]),
                "k2": np.ascontiguousarray(k2[NPAIR * i : NPAIR * (i + 1)]),
                "v": np.ascontiguousarray(vq[GPC * i : GPC * (i + 1)]),
                "tri": tri,
            }
        )
    return in_maps


class _Runner:
    """Cached PJRT executor: traces/compiles the NEFF-wrapped jit once,
    keeps the tri constant and the output zero-dummy resident on device,
    and reuses everything across calls."""

    def __init__(self, nc):
        import jax
        from jax.experimental.shard_map import shard_map
        from jax.sharding import Mesh, PartitionSpec

        from concourse import bass2jax as b2j

        b2j.install_neuronx_cc_hook()
        self._jax = jax
        self.nc = nc
        part_name = nc.partition_id_tensor.name if nc.partition_id_tensor else None
        in_names, out_names, out_avals = [], [], []
        for alloc in nc.m.functions[0].allocations:
            if not isinstance(alloc, mybir.MemoryLocationSet):
                continue
            name = alloc.memorylocations[0].name
            if alloc.kind == "ExternalInput":
                if name != part_name:
                    in_names.append(name)
            elif alloc.kind == "ExternalOutput":
                out_names.append(name)
                shape = tuple(alloc.tensor_shape)
                dtype = mybir.dt.np(alloc.dtype)
                out_avals.append(jax.core.ShapedArray(shape, dtype))
        self.in_names, self.out_names = in_names, out_names
        n_params, n_outs = len(in_names), len(out_names)
        all_names = in_names + out_names
        if part_name is not None:
            all_names = all_names + [part_name]

        def _body(*args):
            operands = list(args)
            if part_name is not None:
                operands.append(b2j.partition_id_tensor())
            return tuple(
                b2j._bass_exec_p.bind(
                    *operands,
                    out_avals=tuple(out_avals),
                    in_names=tuple(all_names),
                    out_names=tuple(out_names),
                    lowering_input_output_aliases=(),
                    sim_require_finite=True,
                    sim_require_nnan=True,
                    nc=nc,
                )
            )

        devices = jax.devices()[:NCORES]
        mesh = Mesh(np.asarray(devices), ("core",))
        self.mesh = mesh
        self.sharding = jax.sharding.NamedSharding(mesh, PartitionSpec("core"))
        self.jitted = jax.jit(
            shard_map(
                _body,
                mesh=mesh,
                in_specs=(PartitionSpec("core"),) * (n_params + n_outs),
                out_specs=(PartitionSpec("core"),) * n_outs,
                check_rep=False,
            ),
            keep_unused=True,
        )
        assert self.in_names == ["q2", "k2", "v", "tri"], self.in_names
        assert self.out_names == ["out"], self.out_names
        # persistent device-resident constants (transferred once)
        self.d_tri = jax.device_put(np.tile(_tri_np(), (NCORES, 1)), self.sharding)
        self.d_zero_out = jax.device_put(
            np.zeros((NCORES * GPC, T * E + 256), np.int8), self.sharding
        )
        from concurrent.futures import ThreadPoolExecutor

        self.pool = ThreadPoolExecutor(4)
        self.q2_buf = np.empty((BH // 2, T, 2 * E), np.float16)
        self.k2_buf = np.empty((BH // 2, T, 2 * E), np.float16)
        self.vbuf = np.empty((BH, T * E + 512), np.int8)
        self.vtmp = np.empty((BH, T, E), np.float32)

    def put(self, arr):
        return self._jax.device_put(arr, self.sharding)

    def pack_qk_fast(self, x, buf):
        # [4, 8, T, E] fp32 -> [16 pairs, T, 128] fp16, 4 threads
        xr = np.asarray(x).reshape(BH // 2, 2, T, E)

        def fill(i0, i1):
            buf[i0:i1, :, :E] = xr[i0:i1, 0]
            buf[i0:i1, :, E:] = xr[i0:i1, 1]

        futs = [self.pool.submit(fill, 4 * i, 4 * i + 4) for i in range(4)]
        for f in futs:
            f.result()
        return buf

    def quant_v_fast(self, vr):
        # _quant_v parallelized over head slices (numpy ufuncs release the GIL)
        def qslice(h0, h1):
            _quant_v(vr[h0:h1], self.vbuf[h0:h1], self.vtmp[h0:h1])

        futs = [self.pool.submit(qslice, 8 * i, 8 * i + 8) for i in range(4)]
        for f in futs:
            f.result()
        return self.vbuf

    def call_device(self, dq2, dk2, dv):
        (out,) = self.jitted(dq2, dk2, dv, self.d_tri, self.d_zero_out)
        return out


def _get_runner():
    if "runner" not in _CACHE:
        _CACHE["runner"] = _Runner(_build())
    return _CACHE["runner"]


def _dequant_into(res, dst):
    # res [nh, T*E+256] int8 -> dst [nh, T, E] fp32 (single fused pass)
    nh = res.shape[0]
    m = res[:, T * E :].reshape(nh, WS, 2).copy().view(np.float16)
    np.multiply(
        res[:, : T * E].reshape(nh, NW, WS, E),
        m.astype(np.float32).reshape(nh, 1, WS, 1) / QMAX,
        out=dst.reshape(nh, NW, WS, E),
    )


def kernel(q, k, v):
    r = _get_runner()
    # pack + upload; device_put dispatches async so the next pack overlaps
    # the previous transfer
    dq = r.put(r.pack_qk_fast(q, r.q2_buf))
    dk = r.put(r.pack_qk_fast(k, r.k2_buf))
    vr = np.asarray(v, dtype=np.float32).reshape(BH, T, E)
    dv = r.put(r.quant_v_fast(vr))
    out = r.call_device(dq, dk, dv)
    full = np.empty((BH, T, E), np.float32)
    _dequant_into(np.asarray(out), full)
    return full.reshape(B, H, T, E)


def run(q, k, v, **kw):
    return kernel(q, k, v), None


# revision 50
# speedup vs baseline: 1.5080x; 1.2004x over previous
"""Local (windowed, causal) attention on 8 TRN2 NeuronCores.

Shapes (hardcoded): q,k,v [4, 8, 4096, 64] fp32, window=128, look_backward=1.
Sharding: merged batch*heads axis (32) -> 4 heads per core, data parallel.

The end-to-end call is dominated by the host<->device tunnel (~65 MB/s up,
~47 MB/s down), so the wire format is fp16 and all layout work happens on
device:
  - q, k ship as head-PAIR packed [pairs, T, 128] fp16 (cols = 2 heads x 64),
    one strided astype pass on host; the e-major transpose happens on device
    via the DMA XBAR transpose (16x128 tiles, ~14 ns/tile).
  - v ships natural [heads, T, 64] fp16; the softmax-denominator ones column
    is memset on device.
  - out comes back int8, quantized per output partition against its abs-max;
    the fp16 scales ride in the same tensor (bitcast), so the host dequant
    multiply by m/QMAX is the exact inverse of the device quant.
  - tri mask constant and the output zero-dummy live on device permanently.

Device algorithm per head pair, per key-window c (32 windows of 128 tokens):
  S^T = K_c^T . [Q_c | Q_{c+1}]      (one matmul per head; the two heads of a
                                      pair sit in PE row groups 0-63 / 64-127
                                      and overlap in the array)
  P^T = exp(scale * S^T)             (ACT, PSUM->SBUF, fp16)
  P^T[:, :128] *= tri                (GpSimd, causal mask on diagonal block)
  O_w += P^T_block . [V_c | 1]       (two matmuls accumulate the two key-window
                                      contributions per query window; the ones
                                      column accumulates the softmax denom)
  out_w = O_w[:, :64] * 1/O_w[:, 64] (DVE reciprocal + tensor_scalar_mul)
"""

import numpy as np

import concourse.bass as bass
import concourse.tile as tile
from concourse import bacc, mybir

B, H, T, E = 4, 8, 4096, 64
BH = B * H
WS = 128                      # window size
NW = T // WS                  # 32 windows per sequence
NCORES = 8
GPC = BH // NCORES            # 4 heads per core
NPAIR = GPC // 2              # 2 head pairs per core
SCALE = float(E) ** -0.5
F32 = mybir.dt.float32
F16 = mybir.dt.float16
I8 = mybir.dt.int8
QMAX = 126.0                  # int8 quant range (margin below 127 for safety)


def _emit(tc, qsrcs, ksrcs, vsrcs, tri, out):
    import contextlib

    nc = tc.nc
    Exp = mybir.ActivationFunctionType.Exp
    mult = mybir.AluOpType.mult

    with contextlib.ExitStack() as ctx:
        qk_pool = ctx.enter_context(tc.tile_pool(name="qk", bufs=2))
        qki_pool = ctx.enter_context(tc.tile_pool(name="qki", bufs=2))
        qkf_pool = ctx.enter_context(tc.tile_pool(name="qkf", bufs=2))
        dram_pool = ctx.enter_context(tc.tile_pool(name="scr", bufs=2, space="DRAM"))
        v_pool = ctx.enter_context(tc.tile_pool(name="v", bufs=3))
        vi_pool = ctx.enter_context(tc.tile_pool(name="vi", bufs=3))
        o_sb_pool = ctx.enter_context(tc.tile_pool(name="o_sb", bufs=3))
        p_pool = ctx.enter_context(tc.tile_pool(name="p", bufs=4))
        const_pool = ctx.enter_context(tc.tile_pool(name="const", bufs=1))
        s_pool = ctx.enter_context(tc.tile_pool(name="s", bufs=3, space="PSUM"))
        o_ps_pool = ctx.enter_context(tc.tile_pool(name="o_ps", bufs=5, space="PSUM"))
        r_pool = ctx.enter_context(tc.tile_pool(name="r", bufs=6))

        tri_sb = const_pool.tile([WS, WS], F16)
        nc.sync.dma_start(tri_sb[:], tri[:])

        for pair in range(NPAIR):
            # combined per-head exp scales (SCALE*s_q*s_k, bitcast fp32 in
            # k2's tail): [128, 2], col gg = head gg's scale
            cs_t = r_pool.tile([128, 2], F32, tag="cs", name=f"cs_{pair}")
            nc.sync.dma_start(
                cs_t[:],
                ksrcs[pair][T * 128 : T * 128 + 1024]
                .rearrange("(p b) -> p b", p=WS)
                .bitcast(F32),
            )
            # q/k arrive int8 pair-packed [T, 128]; cast to fp16 (values are
            # +-126 integers, exact in fp16), bounce through a DRAM scratch,
            # then the DMA XBAR transpose gives e-major [128, T] as before
            # (rows 0-63 head0's e, 64-127 head1's e).
            qkT = []
            for src, nm in ((qsrcs[pair], "q"), (ksrcs[pair], "k")):
                i8_t = qki_pool.tile([128, T], I8, tag="i8", name=f"{nm}i_{pair}")
                nc.sync.dma_start(
                    i8_t[:].rearrange("p (a c) -> p a c", c=128),
                    src[: T * 128].rearrange("(a p c) -> p a c", p=WS, c=128),
                )
                f16_t = qkf_pool.tile([128, T], F16, tag="f16", name=f"{nm}f_{pair}")
                nc.gpsimd.tensor_copy(f16_t[:], i8_t[:])
                scr = dram_pool.tile([T, 128], F16, tag="scr", name=f"{nm}s_{pair}")
                nc.sync.dma_start(
                    scr[:].rearrange("(a p) c -> p a c", p=WS),
                    f16_t[:].rearrange("p (a c) -> p a c", c=128),
                )
                tT = qk_pool.tile([128, T], F16, tag=f"{nm}T", name=f"{nm}T_{pair}")
                nc.sync.dma_start_transpose(tT[:], scr[:])
                qkT.append(tT)
            qT_t, kT_t = qkT

            v_t, out_t, ot = [], [], [{}, {}]
            for gg in range(2):
                g = 2 * pair + gg
                # v arrives int8 (per-head scale embedded as bitcast fp32 in
                # the tail); dequantize to fp16 on device
                vi8 = vi_pool.tile([128, NW * E], I8, tag="vi", name=f"vi_{pair}_{gg}")
                nc.sync.dma_start(
                    vi8[:].rearrange("p (w e) -> p w e", e=E),
                    vsrcs[pair][gg][: T * E].rearrange("(w p e) -> p w e", p=WS, e=E),
                )
                vs_t = r_pool.tile([128, 1], F32, tag="vs", name=f"vs_{pair}_{gg}")
                nc.sync.dma_start(
                    vs_t[:],
                    vsrcs[pair][gg][T * E : T * E + 512]
                    .rearrange("(p b) -> p b", p=WS)
                    .bitcast(F32),
                )
                vt = v_pool.tile([128, NW * 65], F16, tag="v", name=f"v_{pair}_{gg}")
                v3 = vt[:].rearrange("p (w e) -> p w e", e=65)
                nc.vector.memset(v3[:, :, 64:65], 1.0)
                nc.vector.tensor_scalar_mul(
                    v3[:, :, 0:64],
                    vi8[:].rearrange("p (w e) -> p w e", e=E),
                    vs_t[:],
                )
                v_t.append(vt)
                outt = o_sb_pool.tile(
                    [128, NW * E], F16, tag="out", name=f"out_{pair}_{gg}"
                )
                out_t.append(outt)

            for c in range(NW):
                n = 256 if c < NW - 1 else 128
                s_t = []
                # both heads' QK^T back-to-back: disjoint PE row groups overlap
                for gg in range(2):
                    p0 = 64 * gg
                    st = s_pool.tile([128, 256], F32, tag="s", name=f"s_{pair}_{gg}_{c}")
                    nc.tensor.matmul(
                        st[:, :n],
                        lhsT=kT_t[p0 : p0 + 64, WS * c : WS * (c + 1)],
                        rhs=qT_t[p0 : p0 + 64, WS * c : WS * c + n],
                        start=True,
                        stop=True,
                    )
                    s_t.append(st)

                for gg in range(2):
                    st, vt, outt, od = s_t[gg], v_t[gg], out_t[gg], ot[gg]
                    p_t = p_pool.tile([128, 256], F16, tag="p", name=f"p_{pair}_{gg}_{c}")
                    nc.scalar.activation(
                        p_t[:, :n], st[:, :n], Exp, scale=cs_t[:, gg : gg + 1]
                    )
                    # causal mask on the diagonal block (keys j valid for i>=j)
                    nc.gpsimd.tensor_tensor(
                        p_t[:, :WS], p_t[:, :WS], tri_sb[:], op=mult
                    )

                    # PV for queries of window c (2nd contribution unless c==0)
                    if c == 0:
                        od[0] = o_ps_pool.tile(
                            [128, 65], F32, tag="o", name=f"o_{pair}_{gg}_0"
                        )
                    nc.tensor.matmul(
                        od[c][:],
                        lhsT=p_t[:, :WS],
                        rhs=vt[:, 65 * c : 65 * c + 65],
                        start=(c == 0),
                        stop=True,
                        skip_group_check=True,
                    )
                    # normalize window c -> SBUF out tile
                    rc = r_pool.tile([128, 1], F32, tag="rc", name=f"rc_{pair}_{gg}_{c}")
                    nc.vector.reciprocal(rc[:], od[c][:, 64:65])
                    nc.vector.tensor_scalar_mul(
                        outt[:, E * c : E * (c + 1)], od[c][:, 0:E], rc[:]
                    )
                    del od[c]

                    # PV for queries of window c+1 (1st contribution)
                    if c < NW - 1:
                        od[c + 1] = o_ps_pool.tile(
                            [128, 65], F32, tag="o", name=f"o_{pair}_{gg}_{c + 1}"
                        )
                        nc.tensor.matmul(
                            od[c + 1][:],
                            lhsT=p_t[:, WS : 2 * WS],
                            rhs=vt[:, 65 * c : 65 * c + 65],
                            start=True,
                            stop=False,
                            skip_group_check=True,
                        )

            for gg in range(2):
                g = 2 * pair + gg
                # int8-quantize against the per-partition abs-max; the fp16
                # scales ride along in the same int8 tensor (bitcast), so the
                # host multiply by m/QMAX is the exact inverse
                m_t = r_pool.tile([128, 1], F16, tag="m", name=f"m_{pair}_{gg}")
                nc.vector.tensor_reduce(
                    m_t[:],
                    out_t[gg][:],
                    axis=mybir.AxisListType.X,
                    op=mybir.AluOpType.max,
                    apply_absolute_value=True,
                )
                s_t = r_pool.tile([128, 1], F32, tag="sc", name=f"sc_{pair}_{gg}")
                nc.vector.reciprocal(s_t[:], m_t[:])
                nc.vector.tensor_scalar_mul(s_t[:], s_t[:], QMAX)
                oi8 = o_sb_pool.tile(
                    [128, NW * E], I8, tag="oi8", name=f"oi8_{pair}_{gg}"
                )
                nc.vector.tensor_scalar_mul(oi8[:], out_t[gg][:], s_t[:])
                nc.sync.dma_start(
                    out[g, : T * E].rearrange("(w p e) -> p w e", p=WS, e=E),
                    oi8[:].rearrange("p (w e) -> p w e", e=E),
                )
                nc.sync.dma_start(
                    out[g, T * E : T * E + 256].rearrange("(p b) -> p b", p=WS),
                    m_t[:].bitcast(I8),
                )


_CACHE = {}


def _build():
    if "nc" in _CACHE:
        return _CACHE["nc"]
    nc = bacc.Bacc(
        "TRN2",
        target_bir_lowering=False,
        debug=False,
        num_devices=NCORES,
    )
    # q/k: int8 pair-packed [T, 128] payload; k2 carries 1024B of bitcast
    # fp32 combined exp scales (SCALE*s_q*s_k per head, replicated x128)
    q2 = nc.dram_tensor("q2", [NPAIR, T * 128], I8, kind="ExternalInput").ap()
    k2 = nc.dram_tensor("k2", [NPAIR, T * 128 + 1024], I8, kind="ExternalInput").ap()
    # per head: T*E int8 payload + 512 bytes of bitcast fp32 dequant scale
    # (the per-head scale replicated over the 128 partitions)
    v = nc.dram_tensor("v", [GPC, T * E + 512], I8, kind="ExternalInput").ap()
    tri = nc.dram_tensor("tri", [WS, WS], F16, kind="ExternalInput").ap()
    # per head: T*E int8 payload + 256 bytes of bitcast fp16 scales
    out = nc.dram_tensor("out", [GPC, T * E + 256], I8, kind="ExternalOutput").ap()

    with tile.TileContext(nc) as tc:
        _emit(tc, [q2[0], q2[1]], [k2[0], k2[1]], [v[0:2], v[2:4]], tri, out)
    nc.compile()
    _CACHE["nc"] = nc
    return nc


def _tri_np():
    # tri[j, i] = 1.0 where query i >= key j (lower-left causal keep mask,
    # stored keys-in-partitions)
    return np.triu(np.ones((WS, WS), dtype=np.float16))


def _headmax(x, tmp=None):
    # per-head abs-max of [32, T, E]
    if tmp is None:
        tmp = np.empty_like(x)
    np.abs(x, out=tmp)
    return tmp.max(axis=(1, 2))


def _quant_pack_qk(xr, s, buf, tmp, pool=None):
    # xr [32, T, E] fp32, s [32] per-head scale -> buf payload [16, T*128] i8
    # (pair-packed: cols 0-63 head0, 64-127 head1, values rint(x/s))
    def work(p0, p1):
        for p in range(p0, p1):
            dst = buf[p, : T * 128].reshape(T, 128)
            for gg in range(2):
                h = 2 * p + gg
                t = tmp[h]
                np.multiply(xr[h], 1.0 / s[h], out=t)
                np.rint(t, out=t)
                dst[:, 64 * gg : 64 * gg + 64] = t
    if pool is None:
        work(0, 16)
    else:
        futs = [pool.submit(work, 4 * i, 4 * i + 4) for i in range(4)]
        for f in futs:
            f.result()
    return buf


def _qk_tail(sq, sk):
    # [16, 1024] int8: per pair [128, 2] fp32 of SCALE*s_q*s_k per head
    cs = (SCALE * sq * sk).astype(np.float32).reshape(16, 1, 2)
    return np.broadcast_to(cs, (16, WS, 2)).reshape(16, WS * 2).copy().view(np.int8)


def _quant_v(vr, vbuf=None, vtmp=None):
    # vr [nh, T, E] fp32 -> [nh, T*E+512] int8: round(v*QMAX/m) payload plus
    # the fp32 dequant scale m/QMAX replicated x128, bitcast into the tail
    nh = vr.shape[0]
    if vbuf is None:
        vbuf = np.empty((nh, T * E + 512), np.int8)
    if vtmp is None:
        vtmp = np.empty((nh, T, E), np.float32)
    np.abs(vr, out=vtmp)
    m = vtmp.max(axis=(1, 2))  # [nh] per-head abs-max
    np.multiply(vr, (QMAX / m)[:, None, None], out=vtmp)
    np.rint(vtmp, out=vtmp)
    vbuf[:, : T * E] = vtmp.reshape(nh, T * E)  # exact cast of integral fp32
    vbuf[:, T * E :] = np.repeat(
        (m / QMAX).astype(np.float32)[:, None], WS, axis=1
    ).view(np.int8)
    return vbuf


def _prep_in_maps(q, k, v):
    """Per-core input dicts (used by the CoreSim gate in test.py)."""
    qr = np.asarray(q, dtype=np.float32).reshape(BH, T, E)
    kr = np.asarray(k, dtype=np.float32).reshape(BH, T, E)
    sq = _headmax(qr) / QMAX
    sk = _headmax(kr) / QMAX
    q2 = _quant_pack_qk(qr, sq, np.empty((16, T * 128), np.int8), np.empty_like(qr))
    k2 = np.empty((16, T * 128 + 1024), np.int8)
    _quant_pack_qk(kr, sk, k2, np.empty_like(kr))
    k2[:, T * 128 :] = _qk_tail(sq, sk)
    vq = _quant_v(np.asarray(v, dtype=np.float32).reshape(BH, T, E))
    tri = _tri_np()
    in_maps = []
    for i in range(NCORES):
        in_maps.append(
            {
                "q2": np.ascontiguousarray(q2[NPAIR * i : NPAIR * (i + 1)]),
                "k2": np.ascontiguousarray(k2[NPAIR * i : NPAIR * (i + 1)]),
                "v": np.ascontiguousarray(vq[GPC * i : GPC * (i + 1)]),
                "tri": tri,
            }
        )
    return in_maps


class _Runner:
    """Cached PJRT executor: traces/compiles the NEFF-wrapped jit once,
    keeps the tri constant and the output zero-dummy resident on device,
    and reuses everything across calls."""

    def __init__(self, nc):
        import jax
        from jax.experimental.shard_map import shard_map
        from jax.sharding import Mesh, PartitionSpec

        from concourse import bass2jax as b2j

        b2j.install_neuronx_cc_hook()
        self._jax = jax
        self.nc = nc
        part_name = nc.partition_id_tensor.name if nc.partition_id_tensor else None
        in_names, out_names, out_avals = [], [], []
        for alloc in nc.m.functions[0].allocations:
            if not isinstance(alloc, mybir.MemoryLocationSet):
                continue
            name = alloc.memorylocations[0].name
            if alloc.kind == "ExternalInput":
                if name != part_name:
                    in_names.append(name)
            elif alloc.kind == "ExternalOutput":
                out_names.append(name)
                shape = tuple(alloc.tensor_shape)
                dtype = mybir.dt.np(alloc.dtype)
                out_avals.append(jax.core.ShapedArray(shape, dtype))
        self.in_names, self.out_names = in_names, out_names
        n_params, n_outs = len(in_names), len(out_names)
        all_names = in_names + out_names
        if part_name is not None:
            all_names = all_names + [part_name]

        def _body(*args):
            operands = list(args)
            if part_name is not None:
                operands.append(b2j.partition_id_tensor())
            return tuple(
                b2j._bass_exec_p.bind(
                    *operands,
                    out_avals=tuple(out_avals),
                    in_names=tuple(all_names),
                    out_names=tuple(out_names),
                    lowering_input_output_aliases=(),
                    sim_require_finite=True,
                    sim_require_nnan=True,
                    nc=nc,
                )
            )

        devices = jax.devices()[:NCORES]
        mesh = Mesh(np.asarray(devices), ("core",))
        self.mesh = mesh
        self.sharding = jax.sharding.NamedSharding(mesh, PartitionSpec("core"))
        self.jitted = jax.jit(
            shard_map(
                _body,
                mesh=mesh,
                in_specs=(PartitionSpec("core"),) * (n_params + n_outs),
                out_specs=(PartitionSpec("core"),) * n_outs,
                check_rep=False,
            ),
            keep_unused=True,
        )
        assert self.in_names == ["q2", "k2", "v", "tri"], self.in_names
        assert self.out_names == ["out"], self.out_names
        # persistent device-resident constants (transferred once)
        self.d_tri = jax.device_put(np.tile(_tri_np(), (NCORES, 1)), self.sharding)
        self.d_zero_out = jax.device_put(
            np.zeros((NCORES * GPC, T * E + 256), np.int8), self.sharding
        )
        from concurrent.futures import ThreadPoolExecutor

        self.pool = ThreadPoolExecutor(4)
        self.q2_buf = np.empty((BH // 2, T * 128), np.int8)
        self.k2_buf = np.empty((BH // 2, T * 128 + 1024), np.int8)
        self.vbuf = np.empty((BH, T * E + 512), np.int8)
        self.vtmp = np.empty((BH, T, E), np.float32)

    def put(self, arr):
        return self._jax.device_put(arr, self.sharding)

    def quant_v_fast(self, vr):
        # _quant_v parallelized over head slices (numpy ufuncs release the GIL)
        def qslice(h0, h1):
            _quant_v(vr[h0:h1], self.vbuf[h0:h1], self.vtmp[h0:h1])

        futs = [self.pool.submit(qslice, 8 * i, 8 * i + 8) for i in range(4)]
        for f in futs:
            f.result()
        return self.vbuf

    def call_device(self, dq2, dk2, dv):
        (out,) = self.jitted(dq2, dk2, dv, self.d_tri, self.d_zero_out)
        return out


def _get_runner():
    if "runner" not in _CACHE:
        _CACHE["runner"] = _Runner(_build())
    return _CACHE["runner"]


def _dequant_into(res, dst):
    # res [nh, T*E+256] int8 -> dst [nh, T, E] fp32 (single fused pass)
    nh = res.shape[0]
    m = res[:, T * E :].reshape(nh, WS, 2).copy().view(np.float16)
    np.multiply(
        res[:, : T * E].reshape(nh, NW, WS, E),
        m.astype(np.float32).reshape(nh, 1, WS, 1) / QMAX,
        out=dst.reshape(nh, NW, WS, E),
    )


def kernel(q, k, v):
    r = _get_runner()
    # quantize + upload; device_put dispatches async so the next pack
    # overlaps the previous transfer
    qr = np.asarray(q, dtype=np.float32).reshape(BH, T, E)
    kr = np.asarray(k, dtype=np.float32).reshape(BH, T, E)
    vr = np.asarray(v, dtype=np.float32).reshape(BH, T, E)
    sq = _headmax(qr, r.vtmp) / QMAX
    dq = r.put(_quant_pack_qk(qr, sq, r.q2_buf, r.vtmp, r.pool))
    sk = _headmax(kr, r.vtmp) / QMAX
    _quant_pack_qk(kr, sk, r.k2_buf, r.vtmp, r.pool)
    r.k2_buf[:, T * 128 :] = _qk_tail(sq, sk)
    dk = r.put(r.k2_buf)
    dv = r.put(r.quant_v_fast(vr))
    out = r.call_device(dq, dk, dv)
    full = np.empty((BH, T, E), np.float32)
    _dequant_into(np.asarray(out), full)
    return full.reshape(B, H, T, E)


def run(q, k, v, **kw):
    return kernel(q, k, v), None


# revision 51
# speedup vs baseline: 1.6573x; 1.0990x over previous
"""Local (windowed, causal) attention on 8 TRN2 NeuronCores.

Shapes (hardcoded): q,k,v [4, 8, 4096, 64] fp32, window=128, look_backward=1.
Sharding: merged batch*heads axis (32) -> 4 heads per core, data parallel.

The end-to-end call is dominated by the host<->device tunnel (~65 MB/s up,
~47 MB/s down), so the wire format is fp16 and all layout work happens on
device:
  - q, k ship as head-PAIR packed [pairs, T, 128] fp16 (cols = 2 heads x 64),
    one strided astype pass on host; the e-major transpose happens on device
    via the DMA XBAR transpose (16x128 tiles, ~14 ns/tile).
  - v ships natural [heads, T, 64] fp16; the softmax-denominator ones column
    is memset on device.
  - out comes back int8, quantized per output partition against its abs-max;
    the fp16 scales ride in the same tensor (bitcast), so the host dequant
    multiply by m/QMAX is the exact inverse of the device quant.
  - tri mask constant and the output zero-dummy live on device permanently.

Device algorithm per head pair, per key-window c (32 windows of 128 tokens):
  S^T = K_c^T . [Q_c | Q_{c+1}]      (one matmul per head; the two heads of a
                                      pair sit in PE row groups 0-63 / 64-127
                                      and overlap in the array)
  P^T = exp(scale * S^T)             (ACT, PSUM->SBUF, fp16)
  P^T[:, :128] *= tri                (GpSimd, causal mask on diagonal block)
  O_w += P^T_block . [V_c | 1]       (two matmuls accumulate the two key-window
                                      contributions per query window; the ones
                                      column accumulates the softmax denom)
  out_w = O_w[:, :64] * 1/O_w[:, 64] (DVE reciprocal + tensor_scalar_mul)
"""

import numpy as np

import concourse.bass as bass
import concourse.tile as tile
from concourse import bacc, mybir

B, H, T, E = 4, 8, 4096, 64
BH = B * H
WS = 128                      # window size
NW = T // WS                  # 32 windows per sequence
NCORES = 8
GPC = BH // NCORES            # 4 heads per core
NPAIR = GPC // 2              # 2 head pairs per core
SCALE = float(E) ** -0.5
F32 = mybir.dt.float32
F16 = mybir.dt.float16
I8 = mybir.dt.int8
QMAX = 126.0                  # int8 quant range (margin below 127 for safety)


def _emit(tc, qsrcs, ksrcs, vsrcs, tri, out):
    import contextlib

    nc = tc.nc
    Exp = mybir.ActivationFunctionType.Exp
    mult = mybir.AluOpType.mult

    with contextlib.ExitStack() as ctx:
        qk_pool = ctx.enter_context(tc.tile_pool(name="qk", bufs=2))
        qki_pool = ctx.enter_context(tc.tile_pool(name="qki", bufs=2))
        qkf_pool = ctx.enter_context(tc.tile_pool(name="qkf", bufs=2))
        dram_pool = ctx.enter_context(tc.tile_pool(name="scr", bufs=2, space="DRAM"))
        v_pool = ctx.enter_context(tc.tile_pool(name="v", bufs=3))
        vi_pool = ctx.enter_context(tc.tile_pool(name="vi", bufs=3))
        o_sb_pool = ctx.enter_context(tc.tile_pool(name="o_sb", bufs=3))
        p_pool = ctx.enter_context(tc.tile_pool(name="p", bufs=4))
        const_pool = ctx.enter_context(tc.tile_pool(name="const", bufs=1))
        s_pool = ctx.enter_context(tc.tile_pool(name="s", bufs=3, space="PSUM"))
        o_ps_pool = ctx.enter_context(tc.tile_pool(name="o_ps", bufs=5, space="PSUM"))
        r_pool = ctx.enter_context(tc.tile_pool(name="r", bufs=6))

        tri_sb = const_pool.tile([WS, WS], F16)
        nc.sync.dma_start(tri_sb[:], tri[:])

        for pair in range(NPAIR):
            # combined per-head exp scales (SCALE*s_q*s_k, bitcast fp32 in
            # k2's tail): [128, 2], col gg = head gg's scale
            cs_t = r_pool.tile([128, 2], F32, tag="cs", name=f"cs_{pair}")
            nc.sync.dma_start(
                cs_t[:],
                ksrcs[pair][T * 128 : T * 128 + 1024]
                .rearrange("(p b) -> p b", p=WS)
                .bitcast(F32),
            )
            # q/k arrive int8 pair-packed [T, 128]; cast to fp16 (values are
            # +-126 integers, exact in fp16), bounce through a DRAM scratch,
            # then the DMA XBAR transpose gives e-major [128, T] as before
            # (rows 0-63 head0's e, 64-127 head1's e).
            qkT = []
            for src, nm in ((qsrcs[pair], "q"), (ksrcs[pair], "k")):
                i8_t = qki_pool.tile([128, T], I8, tag="i8", name=f"{nm}i_{pair}")
                nc.sync.dma_start(
                    i8_t[:].rearrange("p (a c) -> p a c", c=128),
                    src[: T * 128].rearrange("(a p c) -> p a c", p=WS, c=128),
                )
                f16_t = qkf_pool.tile([128, T], F16, tag="f16", name=f"{nm}f_{pair}")
                nc.gpsimd.tensor_copy(f16_t[:], i8_t[:])
                scr = dram_pool.tile([T, 128], F16, tag="scr", name=f"{nm}s_{pair}")
                nc.sync.dma_start(
                    scr[:].rearrange("(a p) c -> p a c", p=WS),
                    f16_t[:].rearrange("p (a c) -> p a c", c=128),
                )
                tT = qk_pool.tile([128, T], F16, tag=f"{nm}T", name=f"{nm}T_{pair}")
                nc.sync.dma_start_transpose(tT[:], scr[:])
                qkT.append(tT)
            qT_t, kT_t = qkT

            v_t, out_t, ot = [], [], [{}, {}]
            for gg in range(2):
                g = 2 * pair + gg
                # v arrives int8 (per-head scale embedded as bitcast fp32 in
                # the tail); dequantize to fp16 on device
                vi8 = vi_pool.tile([128, NW * E], I8, tag="vi", name=f"vi_{pair}_{gg}")
                nc.sync.dma_start(
                    vi8[:].rearrange("p (w e) -> p w e", e=E),
                    vsrcs[pair][gg][: T * E].rearrange("(w p e) -> p w e", p=WS, e=E),
                )
                vs_t = r_pool.tile([128, 1], F32, tag="vs", name=f"vs_{pair}_{gg}")
                nc.sync.dma_start(
                    vs_t[:],
                    vsrcs[pair][gg][T * E : T * E + 512]
                    .rearrange("(p b) -> p b", p=WS)
                    .bitcast(F32),
                )
                vt = v_pool.tile([128, NW * 65], F16, tag="v", name=f"v_{pair}_{gg}")
                v3 = vt[:].rearrange("p (w e) -> p w e", e=65)
                nc.vector.memset(v3[:, :, 64:65], 1.0)
                nc.vector.tensor_scalar_mul(
                    v3[:, :, 0:64],
                    vi8[:].rearrange("p (w e) -> p w e", e=E),
                    vs_t[:],
                )
                v_t.append(vt)
                outt = o_sb_pool.tile(
                    [128, NW * E], F16, tag="out", name=f"out_{pair}_{gg}"
                )
                out_t.append(outt)

            for c in range(NW):
                n = 256 if c < NW - 1 else 128
                s_t = []
                # both heads' QK^T back-to-back: disjoint PE row groups overlap
                for gg in range(2):
                    p0 = 64 * gg
                    st = s_pool.tile([128, 256], F32, tag="s", name=f"s_{pair}_{gg}_{c}")
                    nc.tensor.matmul(
                        st[:, :n],
                        lhsT=kT_t[p0 : p0 + 64, WS * c : WS * (c + 1)],
                        rhs=qT_t[p0 : p0 + 64, WS * c : WS * c + n],
                        start=True,
                        stop=True,
                    )
                    s_t.append(st)

                for gg in range(2):
                    st, vt, outt, od = s_t[gg], v_t[gg], out_t[gg], ot[gg]
                    p_t = p_pool.tile([128, 256], F16, tag="p", name=f"p_{pair}_{gg}_{c}")
                    # scale the integer logits in fp32 on DVE (the ACT-side
                    # scale multiply loses precision at this magnitude)
                    nc.vector.tensor_scalar_mul(
                        st[:, :n], st[:, :n], cs_t[:, gg : gg + 1]
                    )
                    nc.scalar.activation(p_t[:, :n], st[:, :n], Exp, scale=1.0)
                    # causal mask on the diagonal block (keys j valid for i>=j)
                    nc.gpsimd.tensor_tensor(
                        p_t[:, :WS], p_t[:, :WS], tri_sb[:], op=mult
                    )

                    # PV for queries of window c (2nd contribution unless c==0)
                    if c == 0:
                        od[0] = o_ps_pool.tile(
                            [128, 65], F32, tag="o", name=f"o_{pair}_{gg}_0"
                        )
                    nc.tensor.matmul(
                        od[c][:],
                        lhsT=p_t[:, :WS],
                        rhs=vt[:, 65 * c : 65 * c + 65],
                        start=(c == 0),
                        stop=True,
                        skip_group_check=True,
                    )
                    # normalize window c -> SBUF out tile
                    rc = r_pool.tile([128, 1], F32, tag="rc", name=f"rc_{pair}_{gg}_{c}")
                    nc.vector.reciprocal(rc[:], od[c][:, 64:65])
                    nc.vector.tensor_scalar_mul(
                        outt[:, E * c : E * (c + 1)], od[c][:, 0:E], rc[:]
                    )
                    del od[c]

                    # PV for queries of window c+1 (1st contribution)
                    if c < NW - 1:
                        od[c + 1] = o_ps_pool.tile(
                            [128, 65], F32, tag="o", name=f"o_{pair}_{gg}_{c + 1}"
                        )
                        nc.tensor.matmul(
                            od[c + 1][:],
                            lhsT=p_t[:, WS : 2 * WS],
                            rhs=vt[:, 65 * c : 65 * c + 65],
                            start=True,
                            stop=False,
                            skip_group_check=True,
                        )

            for gg in range(2):
                g = 2 * pair + gg
                # int8-quantize against the per-partition abs-max; the fp16
                # scales ride along in the same int8 tensor (bitcast), so the
                # host multiply by m/QMAX is the exact inverse
                m_t = r_pool.tile([128, 1], F16, tag="m", name=f"m_{pair}_{gg}")
                nc.vector.tensor_reduce(
                    m_t[:],
                    out_t[gg][:],
                    axis=mybir.AxisListType.X,
                    op=mybir.AluOpType.max,
                    apply_absolute_value=True,
                )
                s_t = r_pool.tile([128, 1], F32, tag="sc", name=f"sc_{pair}_{gg}")
                nc.vector.reciprocal(s_t[:], m_t[:])
                nc.vector.tensor_scalar_mul(s_t[:], s_t[:], QMAX)
                oi8 = o_sb_pool.tile(
                    [128, NW * E], I8, tag="oi8", name=f"oi8_{pair}_{gg}"
                )
                nc.vector.tensor_scalar_mul(oi8[:], out_t[gg][:], s_t[:])
                nc.sync.dma_start(
                    out[g, : T * E].rearrange("(w p e) -> p w e", p=WS, e=E),
                    oi8[:].rearrange("p (w e) -> p w e", e=E),
                )
                nc.sync.dma_start(
                    out[g, T * E : T * E + 256].rearrange("(p b) -> p b", p=WS),
                    m_t[:].bitcast(I8),
                )


_CACHE = {}


def _build():
    if "nc" in _CACHE:
        return _CACHE["nc"]
    nc = bacc.Bacc(
        "TRN2",
        target_bir_lowering=False,
        debug=False,
        num_devices=NCORES,
    )
    # q/k: int8 pair-packed [T, 128] payload; k2 carries 1024B of bitcast
    # fp32 combined exp scales (SCALE*s_q*s_k per head, replicated x128)
    q2 = nc.dram_tensor("q2", [NPAIR, T * 128], I8, kind="ExternalInput").ap()
    k2 = nc.dram_tensor("k2", [NPAIR, T * 128 + 1024], I8, kind="ExternalInput").ap()
    # per head: T*E int8 payload + 512 bytes of bitcast fp32 dequant scale
    # (the per-head scale replicated over the 128 partitions)
    v = nc.dram_tensor("v", [GPC, T * E + 512], I8, kind="ExternalInput").ap()
    tri = nc.dram_tensor("tri", [WS, WS], F16, kind="ExternalInput").ap()
    # per head: T*E int8 payload + 256 bytes of bitcast fp16 scales
    out = nc.dram_tensor("out", [GPC, T * E + 256], I8, kind="ExternalOutput").ap()

    with tile.TileContext(nc) as tc:
        _emit(tc, [q2[0], q2[1]], [k2[0], k2[1]], [v[0:2], v[2:4]], tri, out)
    nc.compile()
    _CACHE["nc"] = nc
    return nc


def _tri_np():
    # tri[j, i] = 1.0 where query i >= key j (lower-left causal keep mask,
    # stored keys-in-partitions)
    return np.triu(np.ones((WS, WS), dtype=np.float16))


def _headmax(x, tmp=None):
    # per-head abs-max of [32, T, E]
    if tmp is None:
        tmp = np.empty_like(x)
    np.abs(x, out=tmp)
    return tmp.max(axis=(1, 2))


def _quant_pack_qk(xr, s, buf, tmp, pool=None):
    # xr [32, T, E] fp32, s [32] per-head scale -> buf payload [16, T*128] i8
    # (pair-packed: cols 0-63 head0, 64-127 head1, values rint(x/s))
    def work(p0, p1):
        for p in range(p0, p1):
            dst = buf[p, : T * 128].reshape(T, 128)
            for gg in range(2):
                h = 2 * p + gg
                t = tmp[h]
                np.multiply(xr[h], 1.0 / s[h], out=t)
                np.rint(t, out=t)
                dst[:, 64 * gg : 64 * gg + 64] = t
    if pool is None:
        work(0, 16)
    else:
        futs = [pool.submit(work, 4 * i, 4 * i + 4) for i in range(4)]
        for f in futs:
            f.result()
    return buf


def _qk_tail(sq, sk):
    # [16, 1024] int8: per pair [128, 2] fp32 of SCALE*s_q*s_k per head
    cs = (SCALE * sq * sk).astype(np.float32).reshape(16, 1, 2)
    return np.broadcast_to(cs, (16, WS, 2)).reshape(16, WS * 2).copy().view(np.int8)


def _quant_v(vr, vbuf=None, vtmp=None):
    # vr [nh, T, E] fp32 -> [nh, T*E+512] int8: round(v*QMAX/m) payload plus
    # the fp32 dequant scale m/QMAX replicated x128, bitcast into the tail
    nh = vr.shape[0]
    if vbuf is None:
        vbuf = np.empty((nh, T * E + 512), np.int8)
    if vtmp is None:
        vtmp = np.empty((nh, T, E), np.float32)
    np.abs(vr, out=vtmp)
    m = vtmp.max(axis=(1, 2))  # [nh] per-head abs-max
    np.multiply(vr, (QMAX / m)[:, None, None], out=vtmp)
    np.rint(vtmp, out=vtmp)
    vbuf[:, : T * E] = vtmp.reshape(nh, T * E)  # exact cast of integral fp32
    vbuf[:, T * E :] = np.repeat(
        (m / QMAX).astype(np.float32)[:, None], WS, axis=1
    ).view(np.int8)
    return vbuf


def _prep_in_maps(q, k, v):
    """Per-core input dicts (used by the CoreSim gate in test.py)."""
    qr = np.asarray(q, dtype=np.float32).reshape(BH, T, E)
    kr = np.asarray(k, dtype=np.float32).reshape(BH, T, E)
    sq = _headmax(qr) / QMAX
    sk = _headmax(kr) / QMAX
    q2 = _quant_pack_qk(qr, sq, np.empty((16, T * 128), np.int8), np.empty_like(qr))
    k2 = np.empty((16, T * 128 + 1024), np.int8)
    _quant_pack_qk(kr, sk, k2, np.empty_like(kr))
    k2[:, T * 128 :] = _qk_tail(sq, sk)
    vq = _quant_v(np.asarray(v, dtype=np.float32).reshape(BH, T, E))
    tri = _tri_np()
    in_maps = []
    for i in range(NCORES):
        in_maps.append(
            {
                "q2": np.ascontiguousarray(q2[NPAIR * i : NPAIR * (i + 1)]),
                "k2": np.ascontiguousarray(k2[NPAIR * i : NPAIR * (i + 1)]),
                "v": np.ascontiguousarray(vq[GPC * i : GPC * (i + 1)]),
                "tri": tri,
            }
        )
    return in_maps


class _Runner:
    """Cached PJRT executor: traces/compiles the NEFF-wrapped jit once,
    keeps the tri constant and the output zero-dummy resident on device,
    and reuses everything across calls."""

    def __init__(self, nc):
        import jax
        from jax.experimental.shard_map import shard_map
        from jax.sharding import Mesh, PartitionSpec

        from concourse import bass2jax as b2j

        b2j.install_neuronx_cc_hook()
        self._jax = jax
        self.nc = nc
        part_name = nc.partition_id_tensor.name if nc.partition_id_tensor else None
        in_names, out_names, out_avals = [], [], []
        for alloc in nc.m.functions[0].allocations:
            if not isinstance(alloc, mybir.MemoryLocationSet):
                continue
            name = alloc.memorylocations[0].name
            if alloc.kind == "ExternalInput":
                if name != part_name:
                    in_names.append(name)
            elif alloc.kind == "ExternalOutput":
                out_names.append(name)
                shape = tuple(alloc.tensor_shape)
                dtype = mybir.dt.np(alloc.dtype)
                out_avals.append(jax.core.ShapedArray(shape, dtype))
        self.in_names, self.out_names = in_names, out_names
        n_params, n_outs = len(in_names), len(out_names)
        all_names = in_names + out_names
        if part_name is not None:
            all_names = all_names + [part_name]

        def _body(*args):
            operands = list(args)
            if part_name is not None:
                operands.append(b2j.partition_id_tensor())
            return tuple(
                b2j._bass_exec_p.bind(
                    *operands,
                    out_avals=tuple(out_avals),
                    in_names=tuple(all_names),
                    out_names=tuple(out_names),
                    lowering_input_output_aliases=(),
                    sim_require_finite=True,
                    sim_require_nnan=True,
                    nc=nc,
                )
            )

        devices = jax.devices()[:NCORES]
        mesh = Mesh(np.asarray(devices), ("core",))
        self.mesh = mesh
        self.sharding = jax.sharding.NamedSharding(mesh, PartitionSpec("core"))
        self.jitted = jax.jit(
            shard_map(
                _body,
                mesh=mesh,
                in_specs=(PartitionSpec("core"),) * (n_params + n_outs),
                out_specs=(PartitionSpec("core"),) * n_outs,
                check_rep=False,
            ),
            keep_unused=True,
        )
        assert self.in_names == ["q2", "k2", "v", "tri"], self.in_names
        assert self.out_names == ["out"], self.out_names
        # persistent device-resident constants (transferred once)
        self.d_tri = jax.device_put(np.tile(_tri_np(), (NCORES, 1)), self.sharding)
        self.d_zero_out = jax.device_put(
            np.zeros((NCORES * GPC, T * E + 256), np.int8), self.sharding
        )
        from concurrent.futures import ThreadPoolExecutor

        self.pool = ThreadPoolExecutor(4)
        self.q2_buf = np.empty((BH // 2, T * 128), np.int8)
        self.k2_buf = np.empty((BH // 2, T * 128 + 1024), np.int8)
        self.vbuf = np.empty((BH, T * E + 512), np.int8)
        self.vtmp = np.empty((BH, T, E), np.float32)

    def put(self, arr):
        return self._jax.device_put(arr, self.sharding)

    def quant_v_fast(self, vr):
        # _quant_v parallelized over head slices (numpy ufuncs release the GIL)
        def qslice(h0, h1):
            _quant_v(vr[h0:h1], self.vbuf[h0:h1], self.vtmp[h0:h1])

        futs = [self.pool.submit(qslice, 8 * i, 8 * i + 8) for i in range(4)]
        for f in futs:
            f.result()
        return self.vbuf

    def call_device(self, dq2, dk2, dv):
        (out,) = self.jitted(dq2, dk2, dv, self.d_tri, self.d_zero_out)
        return out


def _get_runner():
    if "runner" not in _CACHE:
        _CACHE["runner"] = _Runner(_build())
    return _CACHE["runner"]


def _dequant_into(res, dst):
    # res [nh, T*E+256] int8 -> dst [nh, T, E] fp32 (single fused pass)
    nh = res.shape[0]
    m = res[:, T * E :].reshape(nh, WS, 2).copy().view(np.float16)
    np.multiply(
        res[:, : T * E].reshape(nh, NW, WS, E),
        m.astype(np.float32).reshape(nh, 1, WS, 1) / QMAX,
        out=dst.reshape(nh, NW, WS, E),
    )


def kernel(q, k, v):
    r = _get_runner()
    # quantize + upload; device_put dispatches async so the next pack
    # overlaps the previous transfer
    qr = np.asarray(q, dtype=np.float32).reshape(BH, T, E)
    kr = np.asarray(k, dtype=np.float32).reshape(BH, T, E)
    vr = np.asarray(v, dtype=np.float32).reshape(BH, T, E)
    sq = _headmax(qr, r.vtmp) / QMAX
    dq = r.put(_quant_pack_qk(qr, sq, r.q2_buf, r.vtmp, r.pool))
    sk = _headmax(kr, r.vtmp) / QMAX
    _quant_pack_qk(kr, sk, r.k2_buf, r.vtmp, r.pool)
    r.k2_buf[:, T * 128 :] = _qk_tail(sq, sk)
    dk = r.put(r.k2_buf)
    dv = r.put(r.quant_v_fast(vr))
    out = r.call_device(dq, dk, dv)
    full = np.empty((BH, T, E), np.float32)
    _dequant_into(np.asarray(out), full)
    return full.reshape(B, H, T, E)


def run(q, k, v, **kw):
    return kernel(q, k, v), None


# revision 52
# speedup vs baseline: 1.6871x; 1.0180x over previous
"""Local (windowed, causal) attention on 8 TRN2 NeuronCores.

Shapes (hardcoded): q,k,v [4, 8, 4096, 64] fp32, window=128, look_backward=1.
Sharding: merged batch*heads axis (32) -> 4 heads per core, data parallel.

The end-to-end call is dominated by the host<->device tunnel (~60-145 MB/s
up, ~45 MB/s down, time-varying), so the wire format is int8 everywhere and
all layout work happens on device:
  - q, k ship as head-PAIR packed [T, 128] int8 (cols = 2 heads x 64),
    quantized per head against abs-max/126; the combined exp scales
    (SCALE*s_q*s_k) ride bitcast in k2's tail and are applied to the integer
    logits on DVE in fp32. On device the int8 payload is cast to fp16
    (integers, exact), bounced through a DRAM scratch tile, and transposed
    e-major by the DMA XBAR (16-bit-only, 16x128 tiles).
  - v ships natural [heads, T, 64] int8 with its per-head fp32 dequant scale
    bitcast in the tail; dequantized to fp16 by one tensor_scalar_mul; the
    softmax-denominator ones column is memset on device.
  - out comes back int8, quantized per output partition against its abs-max;
    the fp16 scales ride in the same tensor (bitcast), so the host dequant
    multiply by m/QMAX is the exact inverse of the device quant.
  - tri mask constant and the output zero-dummy live on device permanently.

Device algorithm per head pair, per key-window c (32 windows of 128 tokens):
  S^T = K_c^T . [Q_c | Q_{c+1}]      (one matmul per head; the two heads of a
                                      pair sit in PE row groups 0-63 / 64-127
                                      and overlap in the array)
  P^T = exp(scale * S^T)             (ACT, PSUM->SBUF, fp16)
  P^T[:, :128] *= tri                (GpSimd, causal mask on diagonal block)
  O_w += P^T_block . [V_c | 1]       (two matmuls accumulate the two key-window
                                      contributions per query window; the ones
                                      column accumulates the softmax denom)
  out_w = O_w[:, :64] * 1/O_w[:, 64] (DVE reciprocal + tensor_scalar_mul)
"""

import numpy as np

import concourse.bass as bass
import concourse.tile as tile
from concourse import bacc, mybir

B, H, T, E = 4, 8, 4096, 64
BH = B * H
WS = 128                      # window size
NW = T // WS                  # 32 windows per sequence
NCORES = 8
GPC = BH // NCORES            # 4 heads per core
NPAIR = GPC // 2              # 2 head pairs per core
SCALE = float(E) ** -0.5
F32 = mybir.dt.float32
F16 = mybir.dt.float16
I8 = mybir.dt.int8
QMAX = 126.0                  # int8 quant range (margin below 127 for safety)


def _emit(tc, qsrcs, ksrcs, vsrcs, tri, out):
    import contextlib

    nc = tc.nc
    Exp = mybir.ActivationFunctionType.Exp
    mult = mybir.AluOpType.mult

    with contextlib.ExitStack() as ctx:
        qk_pool = ctx.enter_context(tc.tile_pool(name="qk", bufs=2))
        qki_pool = ctx.enter_context(tc.tile_pool(name="qki", bufs=2))
        qkf_pool = ctx.enter_context(tc.tile_pool(name="qkf", bufs=2))
        dram_pool = ctx.enter_context(tc.tile_pool(name="scr", bufs=2, space="DRAM"))
        v_pool = ctx.enter_context(tc.tile_pool(name="v", bufs=3))
        vi_pool = ctx.enter_context(tc.tile_pool(name="vi", bufs=3))
        o_sb_pool = ctx.enter_context(tc.tile_pool(name="o_sb", bufs=3))
        p_pool = ctx.enter_context(tc.tile_pool(name="p", bufs=4))
        const_pool = ctx.enter_context(tc.tile_pool(name="const", bufs=1))
        s_pool = ctx.enter_context(tc.tile_pool(name="s", bufs=3, space="PSUM"))
        o_ps_pool = ctx.enter_context(tc.tile_pool(name="o_ps", bufs=5, space="PSUM"))
        r_pool = ctx.enter_context(tc.tile_pool(name="r", bufs=6))

        tri_sb = const_pool.tile([WS, WS], F16)
        nc.sync.dma_start(tri_sb[:], tri[:])

        for pair in range(NPAIR):
            # combined per-head exp scales (SCALE*s_q*s_k, bitcast fp32 in
            # k2's tail): [128, 2], col gg = head gg's scale
            cs_t = r_pool.tile([128, 2], F32, tag="cs", name=f"cs_{pair}")
            nc.sync.dma_start(
                cs_t[:],
                ksrcs[pair][T * 128 : T * 128 + 1024]
                .rearrange("(p b) -> p b", p=WS)
                .bitcast(F32),
            )
            # q/k arrive int8 pair-packed [T, 128]; cast to fp16 (values are
            # +-126 integers, exact in fp16), bounce through a DRAM scratch,
            # then the DMA XBAR transpose gives e-major [128, T] as before
            # (rows 0-63 head0's e, 64-127 head1's e).
            qkT = []
            for src, nm in ((qsrcs[pair], "q"), (ksrcs[pair], "k")):
                i8_t = qki_pool.tile([128, T], I8, tag="i8", name=f"{nm}i_{pair}")
                nc.sync.dma_start(
                    i8_t[:].rearrange("p (a c) -> p a c", c=128),
                    src[: T * 128].rearrange("(a p c) -> p a c", p=WS, c=128),
                )
                f16_t = qkf_pool.tile([128, T], F16, tag="f16", name=f"{nm}f_{pair}")
                nc.gpsimd.tensor_copy(f16_t[:], i8_t[:])
                scr = dram_pool.tile([T, 128], F16, tag="scr", name=f"{nm}s_{pair}")
                nc.sync.dma_start(
                    scr[:].rearrange("(a p) c -> p a c", p=WS),
                    f16_t[:].rearrange("p (a c) -> p a c", c=128),
                )
                tT = qk_pool.tile([128, T], F16, tag=f"{nm}T", name=f"{nm}T_{pair}")
                nc.sync.dma_start_transpose(tT[:], scr[:])
                qkT.append(tT)
            qT_t, kT_t = qkT

            v_t, out_t, ot = [], [], [{}, {}]
            for gg in range(2):
                g = 2 * pair + gg
                # v arrives int8 (per-head scale embedded as bitcast fp32 in
                # the tail); dequantize to fp16 on device
                vi8 = vi_pool.tile([128, NW * E], I8, tag="vi", name=f"vi_{pair}_{gg}")
                nc.sync.dma_start(
                    vi8[:].rearrange("p (w e) -> p w e", e=E),
                    vsrcs[pair][gg][: T * E].rearrange("(w p e) -> p w e", p=WS, e=E),
                )
                vs_t = r_pool.tile([128, 1], F32, tag="vs", name=f"vs_{pair}_{gg}")
                nc.sync.dma_start(
                    vs_t[:],
                    vsrcs[pair][gg][T * E : T * E + 512]
                    .rearrange("(p b) -> p b", p=WS)
                    .bitcast(F32),
                )
                vt = v_pool.tile([128, NW * 65], F16, tag="v", name=f"v_{pair}_{gg}")
                v3 = vt[:].rearrange("p (w e) -> p w e", e=65)
                nc.vector.memset(v3[:, :, 64:65], 1.0)
                nc.vector.tensor_scalar_mul(
                    v3[:, :, 0:64],
                    vi8[:].rearrange("p (w e) -> p w e", e=E),
                    vs_t[:],
                )
                v_t.append(vt)
                outt = o_sb_pool.tile(
                    [128, NW * E], F16, tag="out", name=f"out_{pair}_{gg}"
                )
                out_t.append(outt)

            for c in range(NW):
                n = 256 if c < NW - 1 else 128
                s_t = []
                # both heads' QK^T back-to-back: disjoint PE row groups overlap
                for gg in range(2):
                    p0 = 64 * gg
                    st = s_pool.tile([128, 256], F32, tag="s", name=f"s_{pair}_{gg}_{c}")
                    nc.tensor.matmul(
                        st[:, :n],
                        lhsT=kT_t[p0 : p0 + 64, WS * c : WS * (c + 1)],
                        rhs=qT_t[p0 : p0 + 64, WS * c : WS * c + n],
                        start=True,
                        stop=True,
                    )
                    s_t.append(st)

                for gg in range(2):
                    st, vt, outt, od = s_t[gg], v_t[gg], out_t[gg], ot[gg]
                    p_t = p_pool.tile([128, 256], F16, tag="p", name=f"p_{pair}_{gg}_{c}")
                    # scale the integer logits in fp32 on DVE (the ACT-side
                    # scale multiply loses precision at this magnitude)
                    nc.vector.tensor_scalar_mul(
                        st[:, :n], st[:, :n], cs_t[:, gg : gg + 1]
                    )
                    nc.scalar.activation(p_t[:, :n], st[:, :n], Exp, scale=1.0)
                    # causal mask on the diagonal block (keys j valid for i>=j)
                    nc.gpsimd.tensor_tensor(
                        p_t[:, :WS], p_t[:, :WS], tri_sb[:], op=mult
                    )

                    # PV for queries of window c (2nd contribution unless c==0)
                    if c == 0:
                        od[0] = o_ps_pool.tile(
                            [128, 65], F32, tag="o", name=f"o_{pair}_{gg}_0"
                        )
                    nc.tensor.matmul(
                        od[c][:],
                        lhsT=p_t[:, :WS],
                        rhs=vt[:, 65 * c : 65 * c + 65],
                        start=(c == 0),
                        stop=True,
                        skip_group_check=True,
                    )
                    # normalize window c -> SBUF out tile
                    rc = r_pool.tile([128, 1], F32, tag="rc", name=f"rc_{pair}_{gg}_{c}")
                    nc.vector.reciprocal(rc[:], od[c][:, 64:65])
                    nc.vector.tensor_scalar_mul(
                        outt[:, E * c : E * (c + 1)], od[c][:, 0:E], rc[:]
                    )
                    del od[c]

                    # PV for queries of window c+1 (1st contribution)
                    if c < NW - 1:
                        od[c + 1] = o_ps_pool.tile(
                            [128, 65], F32, tag="o", name=f"o_{pair}_{gg}_{c + 1}"
                        )
                        nc.tensor.matmul(
                            od[c + 1][:],
                            lhsT=p_t[:, WS : 2 * WS],
                            rhs=vt[:, 65 * c : 65 * c + 65],
                            start=True,
                            stop=False,
                            skip_group_check=True,
                        )

            for gg in range(2):
                g = 2 * pair + gg
                # int8-quantize against the per-partition abs-max; the fp16
                # scales ride along in the same int8 tensor (bitcast), so the
                # host multiply by m/QMAX is the exact inverse
                m_t = r_pool.tile([128, 1], F16, tag="m", name=f"m_{pair}_{gg}")
                nc.vector.tensor_reduce(
                    m_t[:],
                    out_t[gg][:],
                    axis=mybir.AxisListType.X,
                    op=mybir.AluOpType.max,
                    apply_absolute_value=True,
                )
                s_t = r_pool.tile([128, 1], F32, tag="sc", name=f"sc_{pair}_{gg}")
                nc.vector.reciprocal(s_t[:], m_t[:])
                nc.vector.tensor_scalar_mul(s_t[:], s_t[:], QMAX)
                oi8 = o_sb_pool.tile(
                    [128, NW * E], I8, tag="oi8", name=f"oi8_{pair}_{gg}"
                )
                nc.vector.tensor_scalar_mul(oi8[:], out_t[gg][:], s_t[:])
                nc.sync.dma_start(
                    out[g, : T * E].rearrange("(w p e) -> p w e", p=WS, e=E),
                    oi8[:].rearrange("p (w e) -> p w e", e=E),
                )
                nc.sync.dma_start(
                    out[g, T * E : T * E + 256].rearrange("(p b) -> p b", p=WS),
                    m_t[:].bitcast(I8),
                )


_CACHE = {}


def _build():
    if "nc" in _CACHE:
        return _CACHE["nc"]
    nc = bacc.Bacc(
        "TRN2",
        target_bir_lowering=False,
        debug=False,
        num_devices=NCORES,
    )
    # q/k: int8 pair-packed [T, 128] payload; k2 carries 1024B of bitcast
    # fp32 combined exp scales (SCALE*s_q*s_k per head, replicated x128)
    q2 = nc.dram_tensor("q2", [NPAIR, T * 128], I8, kind="ExternalInput").ap()
    k2 = nc.dram_tensor("k2", [NPAIR, T * 128 + 1024], I8, kind="ExternalInput").ap()
    # per head: T*E int8 payload + 512 bytes of bitcast fp32 dequant scale
    # (the per-head scale replicated over the 128 partitions)
    v = nc.dram_tensor("v", [GPC, T * E + 512], I8, kind="ExternalInput").ap()
    tri = nc.dram_tensor("tri", [WS, WS], F16, kind="ExternalInput").ap()
    # per head: T*E int8 payload + 256 bytes of bitcast fp16 scales
    out = nc.dram_tensor("out", [GPC, T * E + 256], I8, kind="ExternalOutput").ap()

    with tile.TileContext(nc) as tc:
        _emit(tc, [q2[0], q2[1]], [k2[0], k2[1]], [v[0:2], v[2:4]], tri, out)
    nc.compile()
    _CACHE["nc"] = nc
    return nc


def _tri_np():
    # tri[j, i] = 1.0 where query i >= key j (lower-left causal keep mask,
    # stored keys-in-partitions)
    return np.triu(np.ones((WS, WS), dtype=np.float16))


def _headmax(x, tmp=None):
    # per-head abs-max of [32, T, E]
    if tmp is None:
        tmp = np.empty_like(x)
    np.abs(x, out=tmp)
    return tmp.max(axis=(1, 2))


def _quant_pack_qk(xr, s, buf, tmp, pool=None):
    # xr [32, T, E] fp32, s [32] per-head scale -> buf payload [16, T*128] i8
    # (pair-packed: cols 0-63 head0, 64-127 head1, values rint(x/s))
    def work(p0, p1):
        for p in range(p0, p1):
            dst = buf[p, : T * 128].reshape(T, 128)
            for gg in range(2):
                h = 2 * p + gg
                t = tmp[h]
                np.multiply(xr[h], 1.0 / s[h], out=t)
                np.rint(t, out=t)
                dst[:, 64 * gg : 64 * gg + 64] = t
    if pool is None:
        work(0, 16)
    else:
        futs = [pool.submit(work, 4 * i, 4 * i + 4) for i in range(4)]
        for f in futs:
            f.result()
    return buf


def _qk_tail(sq, sk):
    # [16, 1024] int8: per pair [128, 2] fp32 of SCALE*s_q*s_k per head
    cs = (SCALE * sq * sk).astype(np.float32).reshape(16, 1, 2)
    return np.broadcast_to(cs, (16, WS, 2)).reshape(16, WS * 2).copy().view(np.int8)


def _quant_v(vr, vbuf=None, vtmp=None):
    # vr [nh, T, E] fp32 -> [nh, T*E+512] int8: round(v*QMAX/m) payload plus
    # the fp32 dequant scale m/QMAX replicated x128, bitcast into the tail
    nh = vr.shape[0]
    if vbuf is None:
        vbuf = np.empty((nh, T * E + 512), np.int8)
    if vtmp is None:
        vtmp = np.empty((nh, T, E), np.float32)
    np.abs(vr, out=vtmp)
    m = vtmp.max(axis=(1, 2))  # [nh] per-head abs-max
    np.multiply(vr, (QMAX / m)[:, None, None], out=vtmp)
    np.rint(vtmp, out=vtmp)
    vbuf[:, : T * E] = vtmp.reshape(nh, T * E)  # exact cast of integral fp32
    vbuf[:, T * E :] = np.repeat(
        (m / QMAX).astype(np.float32)[:, None], WS, axis=1
    ).view(np.int8)
    return vbuf


def _prep_in_maps(q, k, v):
    """Per-core input dicts (used by the CoreSim gate in test.py)."""
    qr = np.asarray(q, dtype=np.float32).reshape(BH, T, E)
    kr = np.asarray(k, dtype=np.float32).reshape(BH, T, E)
    sq = _headmax(qr) / QMAX
    sk = _headmax(kr) / QMAX
    q2 = _quant_pack_qk(qr, sq, np.empty((16, T * 128), np.int8), np.empty_like(qr))
    k2 = np.empty((16, T * 128 + 1024), np.int8)
    _quant_pack_qk(kr, sk, k2, np.empty_like(kr))
    k2[:, T * 128 :] = _qk_tail(sq, sk)
    vq = _quant_v(np.asarray(v, dtype=np.float32).reshape(BH, T, E))
    tri = _tri_np()
    in_maps = []
    for i in range(NCORES):
        in_maps.append(
            {
                "q2": np.ascontiguousarray(q2[NPAIR * i : NPAIR * (i + 1)]),
                "k2": np.ascontiguousarray(k2[NPAIR * i : NPAIR * (i + 1)]),
                "v": np.ascontiguousarray(vq[GPC * i : GPC * (i + 1)]),
                "tri": tri,
            }
        )
    return in_maps


class _Runner:
    """Cached PJRT executor: traces/compiles the NEFF-wrapped jit once,
    keeps the tri constant and the output zero-dummy resident on device,
    and reuses everything across calls."""

    def __init__(self, nc):
        import jax
        from jax.experimental.shard_map import shard_map
        from jax.sharding import Mesh, PartitionSpec

        from concourse import bass2jax as b2j

        b2j.install_neuronx_cc_hook()
        self._jax = jax
        self.nc = nc
        part_name = nc.partition_id_tensor.name if nc.partition_id_tensor else None
        in_names, out_names, out_avals = [], [], []
        for alloc in nc.m.functions[0].allocations:
            if not isinstance(alloc, mybir.MemoryLocationSet):
                continue
            name = alloc.memorylocations[0].name
            if alloc.kind == "ExternalInput":
                if name != part_name:
                    in_names.append(name)
            elif alloc.kind == "ExternalOutput":
                out_names.append(name)
                shape = tuple(alloc.tensor_shape)
                dtype = mybir.dt.np(alloc.dtype)
                out_avals.append(jax.core.ShapedArray(shape, dtype))
        self.in_names, self.out_names = in_names, out_names
        n_params, n_outs = len(in_names), len(out_names)
        all_names = in_names + out_names
        if part_name is not None:
            all_names = all_names + [part_name]

        def _body(*args):
            operands = list(args)
            if part_name is not None:
                operands.append(b2j.partition_id_tensor())
            return tuple(
                b2j._bass_exec_p.bind(
                    *operands,
                    out_avals=tuple(out_avals),
                    in_names=tuple(all_names),
                    out_names=tuple(out_names),
                    lowering_input_output_aliases=(),
                    sim_require_finite=True,
                    sim_require_nnan=True,
                    nc=nc,
                )
            )

        devices = jax.devices()[:NCORES]
        mesh = Mesh(np.asarray(devices), ("core",))
        self.mesh = mesh
        self.sharding = jax.sharding.NamedSharding(mesh, PartitionSpec("core"))
        self.jitted = jax.jit(
            shard_map(
                _body,
                mesh=mesh,
                in_specs=(PartitionSpec("core"),) * (n_params + n_outs),
                out_specs=(PartitionSpec("core"),) * n_outs,
                check_rep=False,
            ),
            keep_unused=True,
        )
        assert self.in_names == ["q2", "k2", "v", "tri"], self.in_names
        assert self.out_names == ["out"], self.out_names
        # persistent device-resident constants (transferred once)
        self.d_tri = jax.device_put(np.tile(_tri_np(), (NCORES, 1)), self.sharding)
        self.d_zero_out = jax.device_put(
            np.zeros((NCORES * GPC, T * E + 256), np.int8), self.sharding
        )
        from concurrent.futures import ThreadPoolExecutor

        self.pool = ThreadPoolExecutor(4)
        self.q2_buf = np.empty((BH // 2, T * 128), np.int8)
        self.k2_buf = np.empty((BH // 2, T * 128 + 1024), np.int8)
        self.vbuf = np.empty((BH, T * E + 512), np.int8)
        self.vtmp = np.empty((BH, T, E), np.float32)

    def put(self, arr):
        return self._jax.device_put(arr, self.sharding)

    def quant_v_fast(self, vr):
        # _quant_v parallelized over head slices (numpy ufuncs release the GIL)
        def qslice(h0, h1):
            _quant_v(vr[h0:h1], self.vbuf[h0:h1], self.vtmp[h0:h1])

        futs = [self.pool.submit(qslice, 8 * i, 8 * i + 8) for i in range(4)]
        for f in futs:
            f.result()
        return self.vbuf

    def call_device(self, dq2, dk2, dv):
        (out,) = self.jitted(dq2, dk2, dv, self.d_tri, self.d_zero_out)
        return out


def _get_runner():
    if "runner" not in _CACHE:
        _CACHE["runner"] = _Runner(_build())
    return _CACHE["runner"]


def _dequant_into(res, dst):
    # res [nh, T*E+256] int8 -> dst [nh, T, E] fp32 (single fused pass)
    nh = res.shape[0]
    m = res[:, T * E :].reshape(nh, WS, 2).copy().view(np.float16)
    np.multiply(
        res[:, : T * E].reshape(nh, NW, WS, E),
        m.astype(np.float32).reshape(nh, 1, WS, 1) / QMAX,
        out=dst.reshape(nh, NW, WS, E),
    )


def kernel(q, k, v):
    r = _get_runner()
    # quantize + upload; device_put dispatches async so the next pack
    # overlaps the previous transfer
    qr = np.asarray(q, dtype=np.float32).reshape(BH, T, E)
    kr = np.asarray(k, dtype=np.float32).reshape(BH, T, E)
    vr = np.asarray(v, dtype=np.float32).reshape(BH, T, E)
    sq = _headmax(qr, r.vtmp) / QMAX
    dq = r.put(_quant_pack_qk(qr, sq, r.q2_buf, r.vtmp, r.pool))
    sk = _headmax(kr, r.vtmp) / QMAX
    _quant_pack_qk(kr, sk, r.k2_buf, r.vtmp, r.pool)
    r.k2_buf[:, T * 128 :] = _qk_tail(sq, sk)
    dk = r.put(r.k2_buf)
    dv = r.put(r.quant_v_fast(vr))
    out = r.call_device(dq, dk, dv)
    full = np.empty((BH, T, E), np.float32)
    _dequant_into(np.asarray(out), full)
    return full.reshape(B, H, T, E)


def run(q, k, v, **kw):
    return kernel(q, k, v), None
